# revision 16
# baseline (speedup 1.0000x reference)
"""GCN message-passing kernel for Trainium2, 8 NeuronCores (v3).

Math (reference): 3-layer GCN with symmetric normalization and self-loops,
then dot-product decode over label edge pairs.

Key reformulations vs a naive port:
  - A_hat @ (x @ W) == (A_hat @ x) @ W: aggregate first, then dense matmul.
  - Degree normalization is SEPARABLE: nrm_e = dinv[src]*dinv[dst]. The
    src factor is folded into the gather table (rows hold z*dinv), the
    dst factor folds into the per-block output activation
    (relu(c*x) == c*relu(x), c>0). No per-edge weight multiply remains.
  - Aggregation: per dst block, edges are laid out slot-major -- chunk k
    holds the k-th in-edge of every dst (slot == dst_local). dma_gather
    lands chunks as [slot, chunk, feat]; the Tensor engine sums chunks
    into a 4-wide PSUM tile via identity matmuls (512-col rhs windows),
    DVE folds 4->1, PE transposes [dst,feat]->[feat,dst] via an identity
    rhs, and the z matmul + ACT (scale=dinv^2, relu, bf16 cast) finish
    the block. Per-dst overflow beyond the caps goes through spill
    chunks with HOST-PREBUILT one-hot indicators (no on-device indicator
    construction at all).
  - bf16 tables with a zero row head/tail so padding gathers exact zeros.
  - One AllGather per layer replicates the z table; the layer-0 table
    (x*dinv bf16) is computed on the host.

Decode: labels bucketed by (a<32768, b<32768), z3 kept fp32 [N,64];
gather both sides, DVE multiply + reduce, host inverse-permutes.
"""

import numpy as np
import ml_dtypes

P = 128
N_CORES = 8
HEAD = 128          # zero rows at table head
HALF = 32768
LOWN = HALF - HEAD  # src < LOWN -> low table (row = src+HEAD <= 32767)
CAPL = 13           # slot-major chunk cap, low-side
CAPH = 8            # slot-major chunk cap, high-side
GRP = 7             # dst blocks per gather group (7*7=49=bpc)
DEC_SC = 24         # decode sub-call size in 128-label chunks


def _wrap16(flat_idx):
    """dma_gather idx layout: idx i at [i%16, i//16], replicated to 128 rows."""
    t = flat_idx.astype(np.int16).reshape(-1, 16).T  # [16, n/16]
    return np.tile(t, (8, 1))  # [128, n/16]


# ---------------------------------------------------------------- host prep

def prepare_edges(edge_index, n_nodes, bpc):
    npad = N_CORES * bpc * P
    padhi = HEAD + npad - HALF  # zero-tail row index relative to high view

    src = np.asarray(edge_index[0], dtype=np.int64)
    dst = np.asarray(edge_index[1], dtype=np.int64)
    loops = np.arange(n_nodes, dtype=np.int64)
    esrc = np.concatenate([src, loops])
    edst = np.concatenate([dst, loops])

    deg = np.bincount(edst, minlength=npad).astype(np.float64)
    dinv = np.where(deg > 0, 1.0 / np.sqrt(np.maximum(deg, 1.0)), 0.0)

    side = (esrc >= LOWN).astype(np.int64)  # 0 low, 1 high
    key = edst * 2 + side
    order = np.argsort(key, kind="stable")
    sk = key[order]
    ssrc = esrc[order]
    sdst = edst[order]
    starts = np.searchsorted(sk, np.arange(2 * npad))
    rank = np.arange(sk.size) - starts[sk]
    sside = sk & 1

    # slot-major in-cap matrices
    lowmat = np.zeros((npad, CAPL), np.int16)           # pad idx 0 -> zero row
    highmat = np.full((npad, CAPH), padhi, np.int16)    # pad idx -> zero tail
    sel = (sside == 0) & (rank < CAPL)
    lowmat[sdst[sel], rank[sel]] = (ssrc[sel] + HEAD).astype(np.int16)
    sel = (sside == 1) & (rank < CAPH)
    highmat[sdst[sel], rank[sel]] = (ssrc[sel] - LOWN).astype(np.int16)

    # spill edges per (core, block, side)
    cap_arr = np.where(sside == 0, CAPL, CAPH)
    sp = rank >= cap_arr
    sp_dst, sp_src, sp_side = sdst[sp], ssrc[sp], sside[sp]
    sp_blk = sp_dst >> 7
    sp_core = sp_blk // bpc
    sp_bi = sp_blk % bpc
    cnt = np.zeros((N_CORES, bpc, 2), np.int64)
    np.add.at(cnt, (sp_core, sp_bi, sp_side), 1)
    sli = np.ceil(cnt[:, :, 0] / P).astype(np.int64).max(axis=0)  # [bpc]
    shi = np.ceil(cnt[:, :, 1] / P).astype(np.int64).max(axis=0)
    nsl, nsh = int(sli.sum()), int(shi.sum())
    NS = nsl + nsh

    # spill streams per core: low chunks for all blocks (block order), then high
    sp_idx = np.zeros((N_CORES, max(NS, 1) * P), np.int16)
    sp_idx[:, nsl * P:] = padhi
    sind = np.zeros((N_CORES, P, max(NS, 1) * P), ml_dtypes.bfloat16)
    lo_off = np.concatenate([[0], np.cumsum(sli)])
    hi_off = np.concatenate([[0], np.cumsum(shi)])
    ordsp = np.lexsort((sp_dst, sp_bi, sp_core))
    sp_dst, sp_src, sp_side = sp_dst[ordsp], sp_src[ordsp], sp_side[ordsp]
    sp_core, sp_bi = sp_core[ordsp], sp_bi[ordsp]
    for c in range(N_CORES):
        m = sp_core == c
        d, s, sd, bi = sp_dst[m], sp_src[m], sp_side[m], sp_bi[m]
        for b in range(bpc):
            mb = bi == b
            dl = (d[mb] & 127).astype(np.int64)
            srcb, sdb = s[mb], sd[mb]
            lo = sdb == 0
            nl = int(lo.sum())
            pos = int(lo_off[b]) * P
            sp_idx[c, pos:pos + nl] = (srcb[lo] + HEAD).astype(np.int16)
            jj = pos + np.arange(nl)
            sind[c][jj % P, (jj // P) * P + dl[lo]] = 1.0
            nh = int((~lo).sum())
            pos = (nsl + int(hi_off[b])) * P
            sp_idx[c, pos:pos + nh] = (srcb[~lo] - LOWN).astype(np.int16)
            jj = pos + np.arange(nh)
            sind[c][jj % P, (jj // P) * P + dl[~lo]] = 1.0

    # main gather idx stream per core: per group of GRP blocks, low then high
    eidx = []
    for c in range(N_CORES):
        parts = []
        for g in range(bpc // GRP):
            rows = np.arange((c * bpc + g * GRP) * P, (c * bpc + (g + 1) * GRP) * P)
            lm = lowmat[rows].reshape(GRP, P, CAPL).transpose(0, 2, 1)
            parts.append(lm.ravel())
            hm = highmat[rows].reshape(GRP, P, CAPH).transpose(0, 2, 1)
            parts.append(hm.ravel())
        eidx.append(_wrap16(np.concatenate(parts)))
    eidx = np.stack(eidx)

    spidx = np.stack([_wrap16(sp_idx[c]) for c in range(N_CORES)])

    dpc = dinv.reshape(N_CORES, bpc, P)
    dinv1 = np.ascontiguousarray(dpc.transpose(0, 2, 1)).astype(np.float32)
    dinv2 = (dinv1 ** 2).astype(np.float32)

    return dict(eidx=eidx, spidx=spidx, sind=sind, sli=sli, shi=shi,
                nsl=nsl, nsh=nsh, dinv=dinv, dinv1=dinv1, dinv2=dinv2)


def prepare_labels(edge_label_index, n_label):
    """Bucket labels by (a<HALF, b<HALF) per core, pad to 128 multiples."""
    a = np.asarray(edge_label_index[0], dtype=np.int64)
    b = np.asarray(edge_label_index[1], dtype=np.int64)
    per = n_label // N_CORES
    buckets_per_core = []
    for c in range(N_CORES):
        la = a[c * per:(c + 1) * per]
        lb = b[c * per:(c + 1) * per]
        lab = np.arange(c * per, (c + 1) * per)
        bid = (la >= HALF) * 2 + (lb >= HALF)
        buckets_per_core.append([(la[bid == k], lb[bid == k], lab[bid == k])
                                 for k in range(4)])
    tcnt = [max(int(np.ceil(len(buckets_per_core[c][k][0]) / P))
                for c in range(N_CORES)) for k in range(4)]
    T = sum(tcnt)
    aidx = np.zeros((N_CORES, T * P), np.int64)
    bidx = np.zeros((N_CORES, T * P), np.int64)
    labmap = np.full((N_CORES, T * P), -1, np.int64)
    for c in range(N_CORES):
        pos = 0
        for k in range(4):
            la, lb, lab = buckets_per_core[c][k]
            n = len(la)
            cap = tcnt[k] * P
            aidx[c, pos:pos + n] = la - (HALF if k >= 2 else 0)
            bidx[c, pos:pos + n] = lb - (HALF if k % 2 else 0)
            labmap[c, pos:pos + n] = lab
            pos += cap
    la_s = np.stack([_wrap16(aidx[c]) for c in range(N_CORES)])
    lb_s = np.stack([_wrap16(bidx[c]) for c in range(N_CORES)])
    return dict(la=la_s, lb=lb_s, tcnt=tcnt, T=T, labmap=labmap)


# ------------------------------------------------------------- device kernel

def build_bass(n_nodes, bpc, sli, shi, nsl, nsh, tcnt, in_c, hid_c, out_c,
               bias_zero=True):
    from concourse import bacc, bass, mybir
    import concourse.tile as tile

    NPAD = N_CORES * bpc * P
    NROWS = HEAD + NPAD + P
    SPC = bpc // GRP  # groups per core
    CNT = CAPL + CAPH
    NS = nsl + nsh
    T = int(sum(tcnt))
    f32 = mybir.dt.float32
    bf16 = mybir.dt.bfloat16
    EIDX_N = bpc * CNT * P

    nc = bacc.Bacc("TRN2", target_bir_lowering=False, debug=False,
                   num_devices=N_CORES, num_swdge_queues=4)

    # inputs
    xd_d = nc.dram_tensor("xd", [bpc * P, in_c], bf16, kind="ExternalInput")
    w_d = [nc.dram_tensor(f"W{i+1}", s, bf16, kind="ExternalInput")
           for i, s in enumerate([[in_c, hid_c], [hid_c, hid_c], [hid_c, out_c]])]
    b_d = [nc.dram_tensor(f"b{i+1}", [s], f32, kind="ExternalInput")
           for i, s in enumerate([hid_c, hid_c, out_c])]
    eidx_d = nc.dram_tensor("eidx", [P, EIDX_N // 16], mybir.dt.int16,
                            kind="ExternalInput")
    spidx_d = nc.dram_tensor("spidx", [P, max(NS * P // 16, 16)],
                             mybir.dt.int16, kind="ExternalInput")
    sind_d = nc.dram_tensor("sind", [P, max(NS, 1) * P], bf16,
                            kind="ExternalInput")
    dinv1_d = nc.dram_tensor("dinv1", [P, bpc], f32, kind="ExternalInput")
    dinv2_d = nc.dram_tensor("dinv2", [P, bpc], f32, kind="ExternalInput")
    la_d = nc.dram_tensor("la", [P, T * P // 16], mybir.dt.int16,
                          kind="ExternalInput")
    lb_d = nc.dram_tensor("lb", [P, T * P // 16], mybir.dt.int16,
                          kind="ExternalInput")
    out_d = nc.dram_tensor("out", [P, T], f32, kind="ExternalOutput")

    # internal DRAM
    t_d = [nc.dram_tensor(f"T{l}", [NROWS, in_c], bf16, kind="Internal",
                          addr_space="Shared") for l in range(3)]
    zs_d = [nc.dram_tensor(f"zs{l}", [bpc * P, hid_c], bf16, kind="Internal")
            for l in range(2)]
    xs_d = nc.dram_tensor("xs", [bpc * P, in_c], bf16, kind="Internal")
    zs3_d = nc.dram_tensor("zs3", [bpc * P, out_c], f32, kind="Internal")
    z3_d = nc.dram_tensor("z3f", [NPAD, out_c], f32, kind="Internal",
                          addr_space="Shared")

    gq = [0]

    def next_q():
        q = gq[0]
        gq[0] = (q + 1) % 4
        return q

    lo_off = np.concatenate([[0], np.cumsum(sli)]).astype(int)
    hi_off = np.concatenate([[0], np.cumsum(shi)]).astype(int)

    with tile.TileContext(nc) as tc:
        with (
            tc.tile_pool(name="consts", bufs=1) as cst,
            tc.tile_pool(name="gath", bufs=2) as gp,
            tc.tile_pool(name="spill", bufs=1) as sp,
            tc.tile_pool(name="work", bufs=4) as wp,
            tc.tile_pool(name="dec", bufs=2) as dp,
            tc.tile_pool(name="psum", bufs=2, space="PSUM") as ps,
        ):
            # ---- constants
            ident = cst.tile([P, P], bf16)
            nc.gpsimd.memset(ident[:], 0.0)
            nc.gpsimd.affine_select(
                out=ident[:], in_=ident[:],
                compare_op=mybir.AluOpType.not_equal, fill=1.0,
                base=0, pattern=[[-1, P]], channel_multiplier=1)
            identf = cst.tile([P, P], f32)
            nc.gpsimd.memset(identf[:], 0.0)
            nc.gpsimd.affine_select(
                out=identf[:], in_=identf[:],
                compare_op=mybir.AluOpType.not_equal, fill=1.0,
                base=0, pattern=[[-1, P]], channel_multiplier=1)

            zero_sb = cst.tile([P, in_c], bf16)
            nc.vector.memset(zero_sb[:], 0.0)

            eidx_sb = cst.tile([P, EIDX_N // 16], mybir.dt.int16)
            nc.sync.dma_start(eidx_sb[:], eidx_d[:, :])
            if NS:
                spidx_sb = cst.tile([P, NS * P // 16], mybir.dt.int16)
                nc.sync.dma_start(spidx_sb[:], spidx_d[:, :NS * P // 16])
                sind_sb = cst.tile([P, NS * P], bf16)
                nc.sync.dma_start(sind_sb[:], sind_d[:, :NS * P])
            la_sb = cst.tile([P, T * P // 16], mybir.dt.int16)
            lb_sb = cst.tile([P, T * P // 16], mybir.dt.int16)
            nc.sync.dma_start(la_sb[:], la_d[:, :])
            nc.sync.dma_start(lb_sb[:], lb_d[:, :])
            dinv1_sb = cst.tile([P, bpc], f32)
            dinv2_sb = cst.tile([P, bpc], f32)
            nc.sync.dma_start(dinv1_sb[:], dinv1_d[:, :])
            nc.sync.dma_start(dinv2_sb[:], dinv2_d[:, :])

            w_sb = []
            bfull_sb = []
            if not bias_zero:
                ones_row = cst.tile([1, P], bf16)
                nc.vector.memset(ones_row[:], 1.0)
            for l in range(3):
                oc_l = out_c if l == 2 else hid_c
                wt = cst.tile([hid_c if l else in_c, oc_l], bf16)
                nc.sync.dma_start(wt[:], w_d[l][:, :])
                w_sb.append(wt)
                if not bias_zero:
                    bt = cst.tile([1, oc_l], f32)
                    nc.sync.dma_start(bt[:], b_d[l][None, :])
                    btb = cst.tile([1, oc_l], bf16)
                    nc.vector.tensor_copy(out=btb[:], in_=bt[:])
                    b_ps = ps.tile([P, oc_l], f32, tag="bps", space="PSUM")
                    nc.tensor.matmul(out=b_ps[:], lhsT=ones_row[:], rhs=btb[:],
                                     start=True, stop=True)
                    bft = cst.tile([P, oc_l], f32)
                    nc.vector.tensor_copy(out=bft[:], in_=b_ps[:])
                    bfull_sb.append(bft)

            # ---- zero head/tail rows of each table; allgather x table
            for l in range(3):
                nc.sync.dma_start(t_d[l][0:HEAD, :], zero_sb[:])
                nc.sync.dma_start(t_d[l][HEAD + NPAD:NROWS, :], zero_sb[:])
            nc.sync.dma_start(xs_d[:, :], xd_d[:, :])
            nc.gpsimd.collective_compute(
                "AllGather", mybir.AluOpType.bypass,
                replica_groups=[list(range(N_CORES))],
                ins=[xs_d[:, :]], outs=[t_d[0][HEAD:HEAD + NPAD, :]])

            # ---- 3 GCN layers
            for l in range(3):
                oc = out_c if l == 2 else hid_c
                tab = t_d[l]

                # spill gathers for this layer (one low + one high call)
                if NS:
                    spt = sp.tile([P, NS * in_c], bf16, tag="sp")
                    sp3 = spt[:].rearrange("p (c f) -> p c f", c=NS)
                    if nsl:
                        nc.gpsimd.dma_gather(
                            out_ap=sp3[:, 0:nsl, :] if nsh else sp3,
                            in_ap=tab[:, :],
                            idxs_ap=spidx_sb[:, 0:nsl * 8],
                            num_idxs=nsl * P, num_idxs_reg=nsl * P,
                            elem_size=in_c, single_packet=False,
                            queue_num=next_q())
                    if nsh:
                        nc.gpsimd.dma_gather(
                            out_ap=sp3[:, nsl:, :] if nsl else sp3,
                            in_ap=tab[HALF:, :],
                            idxs_ap=spidx_sb[:, nsl * 8:NS * 8],
                            num_idxs=nsh * P, num_idxs_reg=nsh * P,
                            elem_size=in_c, single_packet=False,
                            queue_num=next_q())

                for g in range(SPC):
                    goff = g * GRP * CNT * P  # idx offset of this group
                    gt = gp.tile([P, GRP * CNT * in_c], bf16, tag="gt")
                    g3 = gt[:].rearrange("p (c f) -> p c f", c=GRP * CNT)
                    lo_n = GRP * CAPL * P
                    hi_n = GRP * CAPH * P
                    nc.gpsimd.dma_gather(
                        out_ap=g3[:, 0:GRP * CAPL, :],
                        in_ap=tab[:, :],
                        idxs_ap=eidx_sb[:, goff // 16:(goff + lo_n) // 16],
                        num_idxs=lo_n, num_idxs_reg=lo_n,
                        elem_size=in_c, single_packet=False,
                        queue_num=next_q())
                    nc.gpsimd.dma_gather(
                        out_ap=g3[:, GRP * CAPL:, :],
                        in_ap=tab[HALF:, :],
                        idxs_ap=eidx_sb[:, (goff + lo_n) // 16:
                                        (goff + lo_n + hi_n) // 16],
                        num_idxs=hi_n, num_idxs_reg=hi_n,
                        elem_size=in_c, single_packet=False,
                        queue_num=next_q())

                    zg = wp.tile([P, GRP * oc], bf16 if l < 2 else f32, tag="zg")
                    for i in range(GRP):
                        b = g * GRP + i
                        # chunk indices of this block inside the group tile
                        chunks = [i * CAPL + k for k in range(CAPL)] + \
                                 [GRP * CAPL + i * CAPH + k for k in range(CAPH)]
                        n_sp = int(sli[b] + shi[b])

                        agg_ps = ps.tile([P, 4 * P], f32, tag="agg", space="PSUM")
                        mms = []  # ("id", out_col0, width, chunk0)|("sp", sc)
                        for w in range((CNT + 3) // 4):
                            cs = chunks[w * 4:(w + 1) * 4]
                            runs = []
                            run = [cs[0]]
                            for cc in cs[1:]:
                                if cc == run[-1] + 1:
                                    run.append(cc)
                                else:
                                    runs.append(run)
                                    run = [cc]
                            runs.append(run)
                            col0 = 0
                            for run in runs:
                                mms.append(("id", col0, len(run), run[0]))
                                col0 += len(run)
                        for kk in range(n_sp):
                            if kk < sli[b]:
                                sc = int(lo_off[b] + kk)
                            else:
                                sc = int(nsl + hi_off[b] + (kk - sli[b]))
                            mms.append(("sp", 0, 1, sc))

                        for mi, (kind, col0, width, src0) in enumerate(mms):
                            last = mi == len(mms) - 1
                            if kind == "id":
                                nc.tensor.matmul(
                                    out=agg_ps[:, col0 * P:(col0 + width) * P],
                                    lhsT=ident[:],
                                    rhs=gt[:, src0 * P:(src0 + width) * P],
                                    start=(mi == 0), stop=last,
                                    skip_group_check=True)
                            else:
                                nc.tensor.matmul(
                                    out=agg_ps[:, 0:P],
                                    lhsT=sind_sb[:, src0 * P:(src0 + 1) * P],
                                    rhs=sp3[:, src0, :],
                                    start=False, stop=last,
                                    skip_group_check=True)

                        # 4 sub-blocks -> agg_sb [dst, feat] fp32
                        agg_sb = wp.tile([P, P], f32, tag="agg_sb")
                        nc.vector.tensor_reduce(
                            out=agg_sb[:],
                            in_=agg_ps[:].rearrange("p (a f) -> p f a", a=4),
                            axis=mybir.AxisListType.X, op=mybir.AluOpType.add)

                        # transpose -> aggT [feat, dst] bf16
                        t_ps = ps.tile([P, P], f32, tag="tp", space="PSUM")
                        nc.tensor.matmul(out=t_ps[:], lhsT=agg_sb[:],
                                         rhs=identf[:], start=True, stop=True)
                        aggt = wp.tile([P, P], bf16, tag="aggt")
                        nc.scalar.activation(
                            out=aggt[:], in_=t_ps[:],
                            func=mybir.ActivationFunctionType.Copy)

                        # z = relu(dinv2 * (agg @ W) + b) etc.
                        z_ps = ps.tile([P, oc], f32, tag="z", space="PSUM")
                        nc.tensor.matmul(out=z_ps[:], lhsT=aggt[:],
                                         rhs=w_sb[l][:], start=True, stop=True)
                        if bias_zero:
                            nc.scalar.activation(
                                out=zg[:, i * oc:(i + 1) * oc], in_=z_ps[:],
                                func=(mybir.ActivationFunctionType.Relu if l < 2
                                      else mybir.ActivationFunctionType.Copy),
                                scale=(dinv2_sb if l < 2 else dinv1_sb)[:, b:b + 1])
                        else:
                            sc_ap = (dinv2_sb if l < 2 else dinv1_sb)[:, b:b + 1]
                            if l < 2:
                                tmp = wp.tile([P, oc], f32, tag="zb")
                                nc.vector.scalar_tensor_tensor(
                                    out=tmp[:], in0=z_ps[:], scalar=sc_ap,
                                    in1=bfull_sb[l][:],
                                    op0=mybir.AluOpType.mult,
                                    op1=mybir.AluOpType.add)
                                nc.scalar.activation(
                                    out=zg[:, i * oc:(i + 1) * oc], in_=tmp[:],
                                    func=mybir.ActivationFunctionType.Relu)
                            else:
                                nc.vector.scalar_tensor_tensor(
                                    out=zg[:, i * oc:(i + 1) * oc],
                                    in0=z_ps[:], scalar=sc_ap,
                                    in1=bfull_sb[l][:],
                                    op0=mybir.AluOpType.mult,
                                    op1=mybir.AluOpType.add)

                    stage = zs_d[l] if l < 2 else zs3_d
                    r0 = g * GRP * P
                    nc.sync.dma_start(
                        stage[r0:r0 + GRP * P, :].rearrange("(g p) f -> p g f",
                                                            g=GRP),
                        zg[:].rearrange("p (g f) -> p g f", g=GRP))

                if l < 2:
                    nc.gpsimd.collective_compute(
                        "AllGather", mybir.AluOpType.bypass,
                        replica_groups=[list(range(N_CORES))],
                        ins=[zs_d[l][:, :]],
                        outs=[t_d[l + 1][HEAD:HEAD + NPAD, :]])
                else:
                    nc.gpsimd.collective_compute(
                        "AllGather", mybir.AluOpType.bypass,
                        replica_groups=[list(range(N_CORES))],
                        ins=[zs3_d[:, :]], outs=[z3_d[:, :]])

            # ---- decode
            res = cst.tile([P, T], f32)
            tbase = 0
            for k in range(4):
                tk = int(tcnt[k])
                if tk == 0:
                    continue
                a_tab = z3_d[HALF:, :] if k >= 2 else z3_d[:, :]
                b_tab = z3_d[HALF:, :] if k % 2 else z3_d[:, :]
                for s0 in range(0, tk, DEC_SC):
                    scn = min(DEC_SC, tk - s0)
                    toff = tbase + s0
                    ga = dp.tile([P, DEC_SC * out_c], f32, tag="ga")
                    gb = dp.tile([P, DEC_SC * out_c], f32, tag="gb")
                    nc.gpsimd.dma_gather(
                        out_ap=ga[:, :scn * out_c].rearrange(
                            "p (c f) -> p c f", c=scn),
                        in_ap=a_tab,
                        idxs_ap=la_sb[:, toff * 8:(toff + scn) * 8],
                        num_idxs=scn * P, num_idxs_reg=scn * P,
                        elem_size=out_c, single_packet=False,
                        queue_num=next_q())
                    nc.gpsimd.dma_gather(
                        out_ap=gb[:, :scn * out_c].rearrange(
                            "p (c f) -> p c f", c=scn),
                        in_ap=b_tab,
                        idxs_ap=lb_sb[:, toff * 8:(toff + scn) * 8],
                        num_idxs=scn * P, num_idxs_reg=scn * P,
                        elem_size=out_c, single_packet=False,
                        queue_num=next_q())
                    nc.vector.tensor_tensor(
                        out=ga[:, :scn * out_c], in0=ga[:, :scn * out_c],
                        in1=gb[:, :scn * out_c], op=mybir.AluOpType.mult)
                    nc.vector.tensor_reduce(
                        out=res[:, toff:toff + scn],
                        in_=ga[:, :scn * out_c].rearrange(
                            "p (c f) -> p c f", c=scn),
                        axis=mybir.AxisListType.X, op=mybir.AluOpType.add)
                tbase += tk
            nc.sync.dma_start(out_d[:, :], res[:])

    nc.finalize()
    return nc


# ---------------------------------------------------------------- entry point

def kernel(x, W1, b1, W2, b2, W3, b3, edge_index, edge_label_index):
    from concourse.bass_utils import run_bass_kernel_spmd

    x = np.ascontiguousarray(np.asarray(x, dtype=np.float32))
    n_nodes, in_c = x.shape
    hid_c = np.asarray(W2).shape[0]
    out_c = np.asarray(W3).shape[1]
    n_label = np.asarray(edge_label_index).shape[1]
    bpc = int(np.ceil(n_nodes / (N_CORES * P)))
    npad = N_CORES * bpc * P

    ed = prepare_edges(edge_index, n_nodes, bpc)
    lb = prepare_labels(edge_label_index, n_label)

    bias_zero = all(np.all(np.asarray(b) == 0) for b in (b1, b2, b3))
    nc = build_bass(n_nodes, bpc, ed["sli"], ed["shi"], ed["nsl"], ed["nsh"],
                    lb["tcnt"], in_c, hid_c, out_c, bias_zero=bias_zero)

    # host-side layer-0 table: x * dinv, bf16, per-core slice
    xp = np.zeros((npad, in_c), np.float32)
    xp[:n_nodes] = x
    xd = (xp * ed["dinv"][:, None]).astype(ml_dtypes.bfloat16)

    common = {
        "W1": np.asarray(W1, np.float32).astype(ml_dtypes.bfloat16),
        "W2": np.asarray(W2, np.float32).astype(ml_dtypes.bfloat16),
        "W3": np.asarray(W3, np.float32).astype(ml_dtypes.bfloat16),
        "b1": np.ascontiguousarray(np.asarray(b1, np.float32)),
        "b2": np.ascontiguousarray(np.asarray(b2, np.float32)),
        "b3": np.ascontiguousarray(np.asarray(b3, np.float32)),
    }
    spc = bpc * P
    in_maps = []
    for c in range(N_CORES):
        m = dict(common)
        m["xd"] = np.ascontiguousarray(xd[c * spc:(c + 1) * spc])
        m["eidx"] = np.ascontiguousarray(ed["eidx"][c])
        m["spidx"] = np.ascontiguousarray(
            ed["spidx"][c] if ed["nsl"] + ed["nsh"] else
            np.zeros((P, 16), np.int16))
        m["sind"] = np.ascontiguousarray(ed["sind"][c])
        m["dinv1"] = np.ascontiguousarray(ed["dinv1"][c])
        m["dinv2"] = np.ascontiguousarray(ed["dinv2"][c])
        m["la"] = np.ascontiguousarray(lb["la"][c])
        m["lb"] = np.ascontiguousarray(lb["lb"][c])
        in_maps.append(m)

    res = run_bass_kernel_spmd(nc, in_maps, core_ids=list(range(N_CORES)))

    out = np.zeros((n_label,), np.float32)
    for c in range(N_CORES):
        o = res.results[c]["out"]  # [P, T]
        flat = o.T.reshape(-1)
        lm = lb["labmap"][c]
        valid = lm >= 0
        out[lm[valid]] = flat[valid]
    return out


# revision 21
# speedup vs baseline: 1.1042x; 1.1042x over previous
"""GCN message-passing kernel for Trainium2, 8 NeuronCores (v4).

Math (reference): 3-layer GCN with symmetric normalization and self-loops,
then dot-product decode over label edge pairs.

Reformulations:
  - A_hat @ (x @ W) == (A_hat @ x) @ W: aggregate, then dense matmul.
  - Degree norm is separable: dinv[src] folds into the gather table
    (rows hold z*dinv), dinv[dst] folds into the output activation
    (relu(c*x) == c*relu(x), c>0). No per-edge weights remain.
  - Aggregation: per dst block, edges are slot-major (chunk k holds the
    k-th in-edge of each dst; slot == dst_local). dma_gather lands
    [slot, chunk, feat] tiles; the Tensor engine sums chunks into a
    4-wide PSUM tile via identity matmuls (512-col windows), DVE folds
    4->1, PE transposes via an identity rhs, then z matmul + ACT
    (scale=dinv^2 + relu + bf16 cast). Per-dst overflow beyond the caps
    goes through spill chunks with host-prebuilt one-hot indicators.
  - bf16 tables with zero head/tail rows so pad slots gather exact zeros.
  - Table rows are GROUP-MAJOR: row(n) = HEAD + g*8*GPA*128 + c*GPA*128
    + j*128 + slot, so one small AllGather per block-group g replicates
    z while later groups still compute (no end-of-layer mega-collective).
    The layer-0 table (x*dinv bf16) is built on the host and fed as an
    input directly -- no startup collective at all.

Decode: labels bucketed by (a<32768, b<32768) on remapped rows, z3 fp32
[N,64]; gather both sides, DVE multiply+reduce, host inverse-permutes.
"""

import numpy as np
import ml_dtypes

P = 128
N_CORES = 8
HEAD = 128          # zero rows at table head
HALF = 32768
CAPL = 13           # slot-major chunk cap, low-side (table row < HALF)
CAPH = 8            # high-side cap
GRP = 3             # dst blocks per gather group
GPA = 7             # dst blocks per allgather group (divides bpc)
DEC_SC = 24         # decode sub-call size in 128-label chunks


def _wrap16(flat_idx):
    t = flat_idx.astype(np.int16).reshape(-1, 16).T  # [16, n/16]
    return np.tile(t, (8, 1))  # [128, n/16]


def _row_map(n, bpc):
    """table row of node n under the group-major layout (HEAD included)."""
    c = n // (bpc * P)
    ln = n % (bpc * P)
    i = ln // P
    g = i // GPA
    j = i % GPA
    return HEAD + g * (N_CORES * GPA * P) + c * (GPA * P) + j * P + (n % P)


# ---------------------------------------------------------------- host prep

def prepare_edges(edge_index, n_nodes, bpc):
    npad = N_CORES * bpc * P
    padhi = HEAD + npad - HALF  # zero-tail row, relative to the high view

    src = np.asarray(edge_index[0], dtype=np.int64)
    dst = np.asarray(edge_index[1], dtype=np.int64)
    loops = np.arange(n_nodes, dtype=np.int64)
    esrc = np.concatenate([src, loops])
    edst = np.concatenate([dst, loops])

    deg = np.bincount(edst, minlength=npad).astype(np.float64)
    dinv = np.where(deg > 0, 1.0 / np.sqrt(np.maximum(deg, 1.0)), 0.0)

    rowmap = _row_map(np.arange(npad, dtype=np.int64), bpc)
    erow = rowmap[esrc]
    side = (erow >= HALF).astype(np.int64)  # 0 low, 1 high
    key = edst * 2 + side
    order = np.argsort(key, kind="stable")
    sk = key[order]
    srow = erow[order]
    sdst = edst[order]
    starts = np.searchsorted(sk, np.arange(2 * npad))
    rank = np.arange(sk.size) - starts[sk]
    sside = sk & 1

    lowmat = np.zeros((npad, CAPL), np.int16)           # pad idx 0 -> zero row
    highmat = np.full((npad, CAPH), padhi, np.int16)    # pad idx -> zero tail
    sel = (sside == 0) & (rank < CAPL)
    lowmat[sdst[sel], rank[sel]] = srow[sel].astype(np.int16)
    sel = (sside == 1) & (rank < CAPH)
    highmat[sdst[sel], rank[sel]] = (srow[sel] - HALF).astype(np.int16)

    # spill edges per (core, block, side)
    cap_arr = np.where(sside == 0, CAPL, CAPH)
    sp = rank >= cap_arr
    sp_dst, sp_row, sp_side = sdst[sp], srow[sp], sside[sp]
    sp_blk = sp_dst >> 7
    sp_core = sp_blk // bpc
    sp_bi = sp_blk % bpc
    cnt = np.zeros((N_CORES, bpc, 2), np.int64)
    np.add.at(cnt, (sp_core, sp_bi, sp_side), 1)
    sli = np.ceil(cnt[:, :, 0] / P).astype(np.int64).max(axis=0)  # [bpc]
    shi = np.ceil(cnt[:, :, 1] / P).astype(np.int64).max(axis=0)
    nsl, nsh = int(sli.sum()), int(shi.sum())
    NS = nsl + nsh

    sp_idx = np.zeros((N_CORES, max(NS, 1) * P), np.int16)
    sp_idx[:, nsl * P:] = padhi
    sind = np.zeros((N_CORES, P, max(NS, 1) * P), ml_dtypes.bfloat16)
    lo_off = np.concatenate([[0], np.cumsum(sli)])
    hi_off = np.concatenate([[0], np.cumsum(shi)])
    ordsp = np.lexsort((sp_dst, sp_bi, sp_core))
    sp_dst, sp_row, sp_side = sp_dst[ordsp], sp_row[ordsp], sp_side[ordsp]
    sp_core, sp_bi = sp_core[ordsp], sp_bi[ordsp]
    for c in range(N_CORES):
        m = sp_core == c
        d, r, sd, bi = sp_dst[m], sp_row[m], sp_side[m], sp_bi[m]
        for b in range(bpc):
            mb = bi == b
            dl = (d[mb] & 127).astype(np.int64)
            rowb, sdb = r[mb], sd[mb]
            lo = sdb == 0
            nl = int(lo.sum())
            pos = int(lo_off[b]) * P
            sp_idx[c, pos:pos + nl] = rowb[lo].astype(np.int16)
            jj = pos + np.arange(nl)
            sind[c][jj % P, (jj // P) * P + dl[lo]] = 1.0
            nh = int((~lo).sum())
            pos = (nsl + int(hi_off[b])) * P
            sp_idx[c, pos:pos + nh] = (rowb[~lo] - HALF).astype(np.int16)
            jj = pos + np.arange(nh)
            sind[c][jj % P, (jj // P) * P + dl[~lo]] = 1.0

    # main idx stream per core: per gather group (GRP blocks), low then high
    groups = [GRP] * (bpc // GRP) + ([bpc % GRP] if bpc % GRP else [])
    eidx = []
    for c in range(N_CORES):
        parts = []
        b0 = 0
        for gs in groups:
            rows = np.arange((c * bpc + b0) * P, (c * bpc + b0 + gs) * P)
            lm = lowmat[rows].reshape(gs, P, CAPL).transpose(0, 2, 1)
            parts.append(lm.ravel())
            hm = highmat[rows].reshape(gs, P, CAPH).transpose(0, 2, 1)
            parts.append(hm.ravel())
            b0 += gs
        eidx.append(_wrap16(np.concatenate(parts)))
    eidx = np.stack(eidx)

    spidx = np.stack([_wrap16(sp_idx[c]) for c in range(N_CORES)])

    dpc = dinv.reshape(N_CORES, bpc, P)
    dinv1 = np.ascontiguousarray(dpc.transpose(0, 2, 1)).astype(np.float32)
    dinv2 = (dinv1 ** 2).astype(np.float32)

    return dict(eidx=eidx, spidx=spidx, sind=sind, sli=sli, shi=shi,
                nsl=nsl, nsh=nsh, dinv=dinv, dinv1=dinv1, dinv2=dinv2,
                rowmap=rowmap, groups=groups)


def prepare_labels(edge_label_index, n_label, rowmap):
    """Bucket labels by (rowa<HALF, rowb<HALF) per core, pad to 128s.

    Uses the remapped z3 rows (rowmap - HEAD)."""
    a = rowmap[np.asarray(edge_label_index[0], dtype=np.int64)] - HEAD
    b = rowmap[np.asarray(edge_label_index[1], dtype=np.int64)] - HEAD
    per = n_label // N_CORES
    buckets_per_core = []
    for c in range(N_CORES):
        la = a[c * per:(c + 1) * per]
        lb = b[c * per:(c + 1) * per]
        lab = np.arange(c * per, (c + 1) * per)
        bid = (la >= HALF) * 2 + (lb >= HALF)
        buckets_per_core.append([(la[bid == k], lb[bid == k], lab[bid == k])
                                 for k in range(4)])
    tcnt = [max(int(np.ceil(len(buckets_per_core[c][k][0]) / P))
                for c in range(N_CORES)) for k in range(4)]
    T = sum(tcnt)
    aidx = np.zeros((N_CORES, T * P), np.int64)
    bidx = np.zeros((N_CORES, T * P), np.int64)
    labmap = np.full((N_CORES, T * P), -1, np.int64)
    for c in range(N_CORES):
        pos = 0
        for k in range(4):
            la, lb, lab = buckets_per_core[c][k]
            n = len(la)
            cap = tcnt[k] * P
            aidx[c, pos:pos + n] = la - (HALF if k >= 2 else 0)
            bidx[c, pos:pos + n] = lb - (HALF if k % 2 else 0)
            labmap[c, pos:pos + n] = lab
            pos += cap
    la_s = np.stack([_wrap16(aidx[c]) for c in range(N_CORES)])
    lb_s = np.stack([_wrap16(bidx[c]) for c in range(N_CORES)])
    return dict(la=la_s, lb=lb_s, tcnt=tcnt, T=T, labmap=labmap)


# ------------------------------------------------------------- device kernel

def build_bass(n_nodes, bpc, groups, sli, shi, nsl, nsh, tcnt,
               in_c, hid_c, out_c, bias_zero=True):
    from concourse import bacc, bass, mybir
    import concourse.tile as tile

    NPAD = N_CORES * bpc * P
    NROWS = HEAD + NPAD + P
    CNT = CAPL + CAPH
    NS = nsl + nsh
    T = int(sum(tcnt))
    NAG = bpc // GPA  # allgather groups per core
    f32 = mybir.dt.float32
    bf16 = mybir.dt.bfloat16
    EIDX_N = bpc * CNT * P

    nc = bacc.Bacc("TRN2", target_bir_lowering=False, debug=False,
                   num_devices=N_CORES, num_swdge_queues=4)

    # inputs
    t0_d = nc.dram_tensor("xt", [NROWS, in_c], bf16, kind="ExternalInput")
    w_d = [nc.dram_tensor(f"W{i+1}", s, bf16, kind="ExternalInput")
           for i, s in enumerate([[in_c, hid_c], [hid_c, hid_c], [hid_c, out_c]])]
    b_d = [nc.dram_tensor(f"b{i+1}", [s], f32, kind="ExternalInput")
           for i, s in enumerate([hid_c, hid_c, out_c])]
    eidx_d = nc.dram_tensor("eidx", [P, EIDX_N // 16], mybir.dt.int16,
                            kind="ExternalInput")
    spidx_d = nc.dram_tensor("spidx", [P, max(NS * P // 16, 16)],
                             mybir.dt.int16, kind="ExternalInput")
    sind_d = nc.dram_tensor("sind", [P, max(NS, 1) * P], bf16,
                            kind="ExternalInput")
    dinv1_d = nc.dram_tensor("dinv1", [P, bpc], f32, kind="ExternalInput")
    dinv2_d = nc.dram_tensor("dinv2", [P, bpc], f32, kind="ExternalInput")
    la_d = nc.dram_tensor("la", [P, T * P // 16], mybir.dt.int16,
                          kind="ExternalInput")
    lb_d = nc.dram_tensor("lb", [P, T * P // 16], mybir.dt.int16,
                          kind="ExternalInput")
    out_d = nc.dram_tensor("out", [P, T], f32, kind="ExternalOutput")

    # internal DRAM
    t_d = [t0_d] + [nc.dram_tensor(f"T{l}", [NROWS, in_c], bf16,
                                   kind="Internal", addr_space="Shared")
                    for l in (1, 2)]
    zs_d = [nc.dram_tensor(f"zs{l}", [bpc * P, hid_c], bf16, kind="Internal")
            for l in range(2)]
    zs3_d = nc.dram_tensor("zs3", [bpc * P, out_c], f32, kind="Internal")
    z3_d = nc.dram_tensor("z3f", [NPAD, out_c], f32, kind="Internal",
                          addr_space="Shared")

    gq = [0]

    def next_q():
        q = gq[0]
        gq[0] = (q + 1) % 4
        return q

    lo_off = np.concatenate([[0], np.cumsum(sli)]).astype(int)
    hi_off = np.concatenate([[0], np.cumsum(shi)]).astype(int)

    with tile.TileContext(nc) as tc:
        with (
            tc.tile_pool(name="consts", bufs=1) as cst,
            tc.tile_pool(name="gath", bufs=4) as gp,
            tc.tile_pool(name="spill", bufs=1) as sp,
            tc.tile_pool(name="work", bufs=4) as wp,
            tc.tile_pool(name="dec", bufs=2) as dp,
            tc.tile_pool(name="psum", bufs=4, space="PSUM") as ps,
            tc.tile_pool(name="psum2", bufs=2, space="PSUM") as ps2,
        ):
            # ---- constants
            ident = cst.tile([P, P], bf16)
            nc.gpsimd.memset(ident[:], 0.0)
            nc.gpsimd.affine_select(
                out=ident[:], in_=ident[:],
                compare_op=mybir.AluOpType.not_equal, fill=1.0,
                base=0, pattern=[[-1, P]], channel_multiplier=1)
            identf = cst.tile([P, P], f32)
            nc.gpsimd.memset(identf[:], 0.0)
            nc.gpsimd.affine_select(
                out=identf[:], in_=identf[:],
                compare_op=mybir.AluOpType.not_equal, fill=1.0,
                base=0, pattern=[[-1, P]], channel_multiplier=1)

            zero_sb = cst.tile([P, in_c], bf16)
            nc.vector.memset(zero_sb[:], 0.0)

            eidx_sb = cst.tile([P, EIDX_N // 16], mybir.dt.int16)
            nc.sync.dma_start(eidx_sb[:], eidx_d[:, :])
            if NS:
                spidx_sb = cst.tile([P, NS * P // 16], mybir.dt.int16)
                nc.sync.dma_start(spidx_sb[:], spidx_d[:, :NS * P // 16])
                sind_sb = cst.tile([P, NS * P], bf16)
                nc.sync.dma_start(sind_sb[:], sind_d[:, :NS * P])
            la_sb = cst.tile([P, T * P // 16], mybir.dt.int16)
            lb_sb = cst.tile([P, T * P // 16], mybir.dt.int16)
            nc.sync.dma_start(la_sb[:], la_d[:, :])
            nc.sync.dma_start(lb_sb[:], lb_d[:, :])
            dinv1_sb = cst.tile([P, bpc], f32)
            dinv2_sb = cst.tile([P, bpc], f32)
            nc.sync.dma_start(dinv1_sb[:], dinv1_d[:, :])
            nc.sync.dma_start(dinv2_sb[:], dinv2_d[:, :])

            w_sb = []
            bfull_sb = []
            if not bias_zero:
                ones_row = cst.tile([1, P], bf16)
                nc.vector.memset(ones_row[:], 1.0)
            for l in range(3):
                oc_l = out_c if l == 2 else hid_c
                wt = cst.tile([hid_c if l else in_c, oc_l], bf16)
                nc.sync.dma_start(wt[:], w_d[l][:, :])
                w_sb.append(wt)
                if not bias_zero:
                    bt = cst.tile([1, oc_l], f32)
                    nc.sync.dma_start(bt[:], b_d[l][None, :])
                    btb = cst.tile([1, oc_l], bf16)
                    nc.vector.tensor_copy(out=btb[:], in_=bt[:])
                    b_ps = ps2.tile([P, oc_l], f32, tag="bps", space="PSUM")
                    nc.tensor.matmul(out=b_ps[:], lhsT=ones_row[:], rhs=btb[:],
                                     start=True, stop=True)
                    bft = cst.tile([P, oc_l], f32)
                    nc.vector.tensor_copy(out=bft[:], in_=b_ps[:])
                    bfull_sb.append(bft)

            # zero head/tail rows of the z tables (T0 comes pre-zeroed)
            for l in (1, 2):
                nc.sync.dma_start(t_d[l][0:HEAD, :], zero_sb[:])
                nc.sync.dma_start(t_d[l][HEAD + NPAD:NROWS, :], zero_sb[:])

            # ---- 3 GCN layers
            for l in range(3):
                oc = out_c if l == 2 else hid_c
                tab = t_d[l]

                if NS:
                    spt = sp.tile([P, NS * in_c], bf16, tag="sp")
                    sp3 = spt[:].rearrange("p (c f) -> p c f", c=NS)
                    if nsl:
                        nc.gpsimd.dma_gather(
                            out_ap=sp3[:, 0:nsl, :] if nsh else sp3,
                            in_ap=tab[:, :],
                            idxs_ap=spidx_sb[:, 0:nsl * 8],
                            num_idxs=nsl * P, num_idxs_reg=nsl * P,
                            elem_size=in_c, single_packet=False,
                            queue_num=next_q())
                    if nsh:
                        nc.gpsimd.dma_gather(
                            out_ap=sp3[:, nsl:, :] if nsl else sp3,
                            in_ap=tab[HALF:, :],
                            idxs_ap=spidx_sb[:, nsl * 8:NS * 8],
                            num_idxs=nsh * P, num_idxs_reg=nsh * P,
                            elem_size=in_c, single_packet=False,
                            queue_num=next_q())

                stage = zs_d[l] if l < 2 else zs3_d
                zg = None
                goff = 0
                b0 = 0
                for gs in groups:
                    gt = gp.tile([P, GRP * CNT * in_c], bf16, tag="gt")
                    g3 = gt[:].rearrange("p (c f) -> p c f", c=GRP * CNT)
                    lo_n = gs * CAPL * P
                    hi_n = gs * CAPH * P
                    nc.gpsimd.dma_gather(
                        out_ap=g3[:, 0:gs * CAPL, :],
                        in_ap=tab[:, :],
                        idxs_ap=eidx_sb[:, goff // 16:(goff + lo_n) // 16],
                        num_idxs=lo_n, num_idxs_reg=lo_n,
                        elem_size=in_c, single_packet=False,
                        queue_num=next_q())
                    nc.gpsimd.dma_gather(
                        out_ap=g3[:, gs * CAPL:gs * CNT, :],
                        in_ap=tab[HALF:, :],
                        idxs_ap=eidx_sb[:, (goff + lo_n) // 16:
                                        (goff + lo_n + hi_n) // 16],
                        num_idxs=hi_n, num_idxs_reg=hi_n,
                        elem_size=in_c, single_packet=False,
                        queue_num=next_q())
                    goff += lo_n + hi_n

                    for i in range(gs):
                        b = b0 + i
                        j = b % GPA  # position within the allgather group
                        if j == 0:
                            zg = wp.tile([P, GPA * oc],
                                         bf16 if l < 2 else f32, tag="zg")
                        chunks = [i * CAPL + k for k in range(CAPL)] + \
                                 [gs * CAPL + i * CAPH + k for k in range(CAPH)]
                        n_sp = int(sli[b] + shi[b])

                        agg_ps = ps.tile([P, 4 * P], f32, tag="agg", space="PSUM")
                        mms = []
                        for w in range((CNT + 3) // 4):
                            cs = chunks[w * 4:(w + 1) * 4]
                            runs = []
                            run = [cs[0]]
                            for cc in cs[1:]:
                                if cc == run[-1] + 1:
                                    run.append(cc)
                                else:
                                    runs.append(run)
                                    run = [cc]
                            runs.append(run)
                            col0 = 0
                            for run in runs:
                                mms.append(("id", col0, len(run), run[0]))
                                col0 += len(run)
                        for kk in range(n_sp):
                            if kk < sli[b]:
                                sc = int(lo_off[b] + kk)
                            else:
                                sc = int(nsl + hi_off[b] + (kk - sli[b]))
                            mms.append(("sp", 0, 1, sc))

                        for mi, (kind, col0, width, src0) in enumerate(mms):
                            last = mi == len(mms) - 1
                            if kind == "id":
                                nc.tensor.matmul(
                                    out=agg_ps[:, col0 * P:(col0 + width) * P],
                                    lhsT=ident[:],
                                    rhs=gt[:, src0 * P:(src0 + width) * P],
                                    start=(mi == 0), stop=last,
                                    skip_group_check=True)
                            else:
                                nc.tensor.matmul(
                                    out=agg_ps[:, 0:P],
                                    lhsT=sind_sb[:, src0 * P:(src0 + 1) * P],
                                    rhs=sp3[:, src0, :],
                                    start=False, stop=last,
                                    skip_group_check=True)

                        agg_sb = wp.tile([P, P], f32, tag="agg_sb")
                        nc.vector.tensor_reduce(
                            out=agg_sb[:],
                            in_=agg_ps[:].rearrange("p (a f) -> p f a", a=4),
                            axis=mybir.AxisListType.X, op=mybir.AluOpType.add)

                        t_ps = ps2.tile([P, P], f32, tag="tp", space="PSUM")
                        nc.tensor.matmul(out=t_ps[:], lhsT=agg_sb[:],
                                         rhs=identf[:], start=True, stop=True)
                        aggt = wp.tile([P, P], bf16, tag="aggt")
                        nc.scalar.activation(
                            out=aggt[:], in_=t_ps[:],
                            func=mybir.ActivationFunctionType.Copy)

                        z_ps = ps2.tile([P, oc], f32, tag="z", space="PSUM")
                        nc.tensor.matmul(out=z_ps[:], lhsT=aggt[:],
                                         rhs=w_sb[l][:], start=True, stop=True)
                        if bias_zero:
                            nc.scalar.activation(
                                out=zg[:, j * oc:(j + 1) * oc], in_=z_ps[:],
                                func=(mybir.ActivationFunctionType.Relu if l < 2
                                      else mybir.ActivationFunctionType.Copy),
                                scale=(dinv2_sb if l < 2 else dinv1_sb)[:, b:b + 1])
                        else:
                            sc_ap = (dinv2_sb if l < 2 else dinv1_sb)[:, b:b + 1]
                            if l < 2:
                                tmp = wp.tile([P, oc], f32, tag="zb")
                                nc.vector.scalar_tensor_tensor(
                                    out=tmp[:], in0=z_ps[:], scalar=sc_ap,
                                    in1=bfull_sb[l][:],
                                    op0=mybir.AluOpType.mult,
                                    op1=mybir.AluOpType.add)
                                nc.scalar.activation(
                                    out=zg[:, j * oc:(j + 1) * oc], in_=tmp[:],
                                    func=mybir.ActivationFunctionType.Relu)
                            else:
                                nc.vector.scalar_tensor_tensor(
                                    out=zg[:, j * oc:(j + 1) * oc],
                                    in0=z_ps[:], scalar=sc_ap,
                                    in1=bfull_sb[l][:],
                                    op0=mybir.AluOpType.mult,
                                    op1=mybir.AluOpType.add)

                        if j == GPA - 1:
                            # stage this allgather group's z and replicate it
                            ag = b // GPA
                            r0 = ag * GPA * P
                            nc.sync.dma_start(
                                stage[r0:r0 + GPA * P, :].rearrange(
                                    "(g p) f -> p g f", g=GPA),
                                zg[:].rearrange("p (g f) -> p g f", g=GPA))
                            if l < 2:
                                o0 = HEAD + ag * (N_CORES * GPA * P)
                                nc.gpsimd.collective_compute(
                                    "AllGather", mybir.AluOpType.bypass,
                                    replica_groups=[list(range(N_CORES))],
                                    ins=[stage[r0:r0 + GPA * P, :]],
                                    outs=[t_d[l + 1][o0:o0 + N_CORES * GPA * P, :]])
                            else:
                                o0 = ag * (N_CORES * GPA * P)
                                nc.gpsimd.collective_compute(
                                    "AllGather", mybir.AluOpType.bypass,
                                    replica_groups=[list(range(N_CORES))],
                                    ins=[stage[r0:r0 + GPA * P, :]],
                                    outs=[z3_d[o0:o0 + N_CORES * GPA * P, :]])
                    b0 += gs

            # ---- decode
            res = cst.tile([P, T], f32)
            tbase = 0
            for k in range(4):
                tk = int(tcnt[k])
                if tk == 0:
                    continue
                a_tab = z3_d[HALF:, :] if k >= 2 else z3_d[:, :]
                b_tab = z3_d[HALF:, :] if k % 2 else z3_d[:, :]
                for s0 in range(0, tk, DEC_SC):
                    scn = min(DEC_SC, tk - s0)
                    toff = tbase + s0
                    ga = dp.tile([P, DEC_SC * out_c], f32, tag="ga")
                    gb = dp.tile([P, DEC_SC * out_c], f32, tag="gb")
                    nc.gpsimd.dma_gather(
                        out_ap=ga[:, :scn * out_c].rearrange(
                            "p (c f) -> p c f", c=scn),
                        in_ap=a_tab,
                        idxs_ap=la_sb[:, toff * 8:(toff + scn) * 8],
                        num_idxs=scn * P, num_idxs_reg=scn * P,
                        elem_size=out_c, single_packet=False,
                        queue_num=next_q())
                    nc.gpsimd.dma_gather(
                        out_ap=gb[:, :scn * out_c].rearrange(
                            "p (c f) -> p c f", c=scn),
                        in_ap=b_tab,
                        idxs_ap=lb_sb[:, toff * 8:(toff + scn) * 8],
                        num_idxs=scn * P, num_idxs_reg=scn * P,
                        elem_size=out_c, single_packet=False,
                        queue_num=next_q())
                    nc.vector.tensor_tensor(
                        out=ga[:, :scn * out_c], in0=ga[:, :scn * out_c],
                        in1=gb[:, :scn * out_c], op=mybir.AluOpType.mult)
                    nc.vector.tensor_reduce(
                        out=res[:, toff:toff + scn],
                        in_=ga[:, :scn * out_c].rearrange(
                            "p (c f) -> p c f", c=scn),
                        axis=mybir.AxisListType.X, op=mybir.AluOpType.add)
                tbase += tk
            nc.sync.dma_start(out_d[:, :], res[:])

    nc.finalize()
    return nc


# ---------------------------------------------------------------- entry point

def kernel(x, W1, b1, W2, b2, W3, b3, edge_index, edge_label_index):
    from concourse.bass_utils import run_bass_kernel_spmd

    x = np.ascontiguousarray(np.asarray(x, dtype=np.float32))
    n_nodes, in_c = x.shape
    hid_c = np.asarray(W2).shape[0]
    out_c = np.asarray(W3).shape[1]
    n_label = np.asarray(edge_label_index).shape[1]
    bpc = int(np.ceil(n_nodes / (N_CORES * P)))
    npad = N_CORES * bpc * P
    nrows = HEAD + npad + P

    ed = prepare_edges(edge_index, n_nodes, bpc)
    lb = prepare_labels(edge_label_index, n_label, ed["rowmap"])

    bias_zero = all(np.all(np.asarray(b) == 0) for b in (b1, b2, b3))
    nc = build_bass(n_nodes, bpc, ed["groups"], ed["sli"], ed["shi"],
                    ed["nsl"], ed["nsh"], lb["tcnt"], in_c, hid_c, out_c,
                    bias_zero=bias_zero)

    # host-side layer-0 table: x * dinv, bf16, group-major rows, zero pads
    xp = np.zeros((npad, in_c), np.float32)
    xp[:n_nodes] = x
    xt = np.zeros((nrows, in_c), ml_dtypes.bfloat16)
    xt[ed["rowmap"]] = (xp * ed["dinv"][:, None]).astype(ml_dtypes.bfloat16)

    common = {
        "xt": xt,
        "W1": np.asarray(W1, np.float32).astype(ml_dtypes.bfloat16),
        "W2": np.asarray(W2, np.float32).astype(ml_dtypes.bfloat16),
        "W3": np.asarray(W3, np.float32).astype(ml_dtypes.bfloat16),
        "b1": np.ascontiguousarray(np.asarray(b1, np.float32)),
        "b2": np.ascontiguousarray(np.asarray(b2, np.float32)),
        "b3": np.ascontiguousarray(np.asarray(b3, np.float32)),
    }
    in_maps = []
    for c in range(N_CORES):
        m = dict(common)
        m["eidx"] = np.ascontiguousarray(ed["eidx"][c])
        m["spidx"] = np.ascontiguousarray(
            ed["spidx"][c] if ed["nsl"] + ed["nsh"] else
            np.zeros((P, 16), np.int16))
        m["sind"] = np.ascontiguousarray(ed["sind"][c])
        m["dinv1"] = np.ascontiguousarray(ed["dinv1"][c])
        m["dinv2"] = np.ascontiguousarray(ed["dinv2"][c])
        m["la"] = np.ascontiguousarray(lb["la"][c])
        m["lb"] = np.ascontiguousarray(lb["lb"][c])
        in_maps.append(m)

    res = run_bass_kernel_spmd(nc, in_maps, core_ids=list(range(N_CORES)))

    out = np.zeros((n_label,), np.float32)
    for c in range(N_CORES):
        o = res.results[c]["out"]  # [P, T]
        flat = o.T.reshape(-1)
        lm = lb["labmap"][c]
        valid = lm >= 0
        out[lm[valid]] = flat[valid]
    return out


# revision 25
# speedup vs baseline: 1.3375x; 1.2112x over previous
"""GCN message-passing kernel for Trainium2, 8 NeuronCores (v4).

Math (reference): 3-layer GCN with symmetric normalization and self-loops,
then dot-product decode over label edge pairs.

Reformulations:
  - A_hat @ (x @ W) == (A_hat @ x) @ W: aggregate, then dense matmul.
  - Degree norm is separable: dinv[src] folds into the gather table
    (rows hold z*dinv), dinv[dst] folds into the output activation
    (relu(c*x) == c*relu(x), c>0). No per-edge weights remain.
  - Aggregation: per dst block, edges are slot-major (chunk k holds the
    k-th in-edge of each dst; slot == dst_local). dma_gather lands
    [slot, chunk, feat] tiles; the Tensor engine sums chunks into a
    4-wide PSUM tile via identity matmuls (512-col windows), DVE folds
    4->1, PE transposes via an identity rhs, then z matmul + ACT
    (scale=dinv^2 + relu + bf16 cast). Per-dst overflow beyond the caps
    goes through spill chunks with host-prebuilt one-hot indicators.
  - bf16 tables with zero head/tail rows so pad slots gather exact zeros.
  - Table rows are GROUP-MAJOR: row(n) = HEAD + g*8*GPA*128 + c*GPA*128
    + j*128 + slot, so one small AllGather per block-group g replicates
    z while later groups still compute (no end-of-layer mega-collective).
    The layer-0 table (x*dinv bf16) is built on the host and fed as an
    input directly -- no startup collective at all.

Decode: labels bucketed by (a<32768, b<32768) on remapped rows, z3 fp32
[N,64]; gather both sides, DVE multiply+reduce, host inverse-permutes.
"""

import numpy as np
import ml_dtypes

P = 128
N_CORES = 8
HEAD = 128          # zero rows at table head
HALF = 32768
CAPL = 13           # slot-major chunk cap, low-side (table row < HALF)
CAPH = 7            # high-side cap
GRP = 3             # dst blocks per gather group
GPA = 49            # dst blocks per allgather group (divides bpc)
DEC_SC = 20         # decode sub-call size in 128-label chunks


def _wrap16(flat_idx):
    t = flat_idx.astype(np.int16).reshape(-1, 16).T  # [16, n/16]
    return np.tile(t, (8, 1))  # [128, n/16]


def _row_map(n, bpc):
    """table row of node n under the group-major layout (HEAD included)."""
    c = n // (bpc * P)
    ln = n % (bpc * P)
    i = ln // P
    g = i // GPA
    j = i % GPA
    return HEAD + g * (N_CORES * GPA * P) + c * (GPA * P) + j * P + (n % P)


# ---------------------------------------------------------------- host prep

def prepare_edges(edge_index, n_nodes, bpc):
    npad = N_CORES * bpc * P
    padhi = HEAD + npad - HALF  # zero-tail row, relative to the high view

    src = np.asarray(edge_index[0], dtype=np.int64)
    dst = np.asarray(edge_index[1], dtype=np.int64)
    loops = np.arange(n_nodes, dtype=np.int64)
    esrc = np.concatenate([src, loops])
    edst = np.concatenate([dst, loops])

    deg = np.bincount(edst, minlength=npad).astype(np.float64)
    dinv = np.where(deg > 0, 1.0 / np.sqrt(np.maximum(deg, 1.0)), 0.0)

    rowmap = _row_map(np.arange(npad, dtype=np.int64), bpc)
    erow = rowmap[esrc]
    side = (erow >= HALF).astype(np.int64)  # 0 low, 1 high
    key = edst * 2 + side
    order = np.argsort(key, kind="stable")
    sk = key[order]
    srow = erow[order]
    sdst = edst[order]
    starts = np.searchsorted(sk, np.arange(2 * npad))
    rank = np.arange(sk.size) - starts[sk]
    sside = sk & 1

    lowmat = np.zeros((npad, CAPL), np.int16)           # pad idx 0 -> zero row
    highmat = np.full((npad, CAPH), padhi, np.int16)    # pad idx -> zero tail
    sel = (sside == 0) & (rank < CAPL)
    lowmat[sdst[sel], rank[sel]] = srow[sel].astype(np.int16)
    sel = (sside == 1) & (rank < CAPH)
    highmat[sdst[sel], rank[sel]] = (srow[sel] - HALF).astype(np.int16)

    # spill edges per (core, block, side)
    cap_arr = np.where(sside == 0, CAPL, CAPH)
    sp = rank >= cap_arr
    sp_dst, sp_row, sp_side = sdst[sp], srow[sp], sside[sp]
    sp_blk = sp_dst >> 7
    sp_core = sp_blk // bpc
    sp_bi = sp_blk % bpc
    cnt = np.zeros((N_CORES, bpc, 2), np.int64)
    np.add.at(cnt, (sp_core, sp_bi, sp_side), 1)
    sli = np.ceil(cnt[:, :, 0] / P).astype(np.int64).max(axis=0)  # [bpc]
    shi = np.ceil(cnt[:, :, 1] / P).astype(np.int64).max(axis=0)
    nsl, nsh = int(sli.sum()), int(shi.sum())
    NS = nsl + nsh

    sp_idx = np.zeros((N_CORES, max(NS, 1) * P), np.int16)
    sp_idx[:, nsl * P:] = padhi
    sind = np.zeros((N_CORES, P, max(NS, 1) * P), ml_dtypes.bfloat16)
    lo_off = np.concatenate([[0], np.cumsum(sli)])
    hi_off = np.concatenate([[0], np.cumsum(shi)])
    ordsp = np.lexsort((sp_dst, sp_bi, sp_core))
    sp_dst, sp_row, sp_side = sp_dst[ordsp], sp_row[ordsp], sp_side[ordsp]
    sp_core, sp_bi = sp_core[ordsp], sp_bi[ordsp]
    for c in range(N_CORES):
        m = sp_core == c
        d, r, sd, bi = sp_dst[m], sp_row[m], sp_side[m], sp_bi[m]
        for b in range(bpc):
            mb = bi == b
            dl = (d[mb] & 127).astype(np.int64)
            rowb, sdb = r[mb], sd[mb]
            lo = sdb == 0
            nl = int(lo.sum())
            pos = int(lo_off[b]) * P
            sp_idx[c, pos:pos + nl] = rowb[lo].astype(np.int16)
            jj = pos + np.arange(nl)
            sind[c][jj % P, (jj // P) * P + dl[lo]] = 1.0
            nh = int((~lo).sum())
            pos = (nsl + int(hi_off[b])) * P
            sp_idx[c, pos:pos + nh] = (rowb[~lo] - HALF).astype(np.int16)
            jj = pos + np.arange(nh)
            sind[c][jj % P, (jj // P) * P + dl[~lo]] = 1.0

    # main idx stream per core: per gather group (GRP blocks), low then high
    groups = [GRP] * (bpc // GRP) + ([bpc % GRP] if bpc % GRP else [])
    eidx = []
    for c in range(N_CORES):
        parts = []
        b0 = 0
        for gs in groups:
            rows = np.arange((c * bpc + b0) * P, (c * bpc + b0 + gs) * P)
            lm = lowmat[rows].reshape(gs, P, CAPL).transpose(0, 2, 1)
            parts.append(lm.ravel())
            hm = highmat[rows].reshape(gs, P, CAPH).transpose(0, 2, 1)
            parts.append(hm.ravel())
            b0 += gs
        eidx.append(_wrap16(np.concatenate(parts)))
    eidx = np.stack(eidx)

    spidx = np.stack([_wrap16(sp_idx[c]) for c in range(N_CORES)])

    dpc = dinv.reshape(N_CORES, bpc, P)
    dinv1 = np.ascontiguousarray(dpc.transpose(0, 2, 1)).astype(np.float32)
    dinv2 = (dinv1 ** 2).astype(np.float32)

    return dict(eidx=eidx, spidx=spidx, sind=sind, sli=sli, shi=shi,
                nsl=nsl, nsh=nsh, dinv=dinv, dinv1=dinv1, dinv2=dinv2,
                rowmap=rowmap, groups=groups)


def prepare_labels(edge_label_index, n_label, rowmap):
    """Bucket labels by (rowa<HALF, rowb<HALF) per core, pad to 128s.

    Uses the remapped z3 rows (rowmap - HEAD)."""
    a = rowmap[np.asarray(edge_label_index[0], dtype=np.int64)] - HEAD
    b = rowmap[np.asarray(edge_label_index[1], dtype=np.int64)] - HEAD
    per = n_label // N_CORES
    buckets_per_core = []
    for c in range(N_CORES):
        la = a[c * per:(c + 1) * per]
        lb = b[c * per:(c + 1) * per]
        lab = np.arange(c * per, (c + 1) * per)
        bid = (la >= HALF) * 2 + (lb >= HALF)
        buckets_per_core.append([(la[bid == k], lb[bid == k], lab[bid == k])
                                 for k in range(4)])
    tcnt = [max(int(np.ceil(len(buckets_per_core[c][k][0]) / P))
                for c in range(N_CORES)) for k in range(4)]
    T = sum(tcnt)
    aidx = np.zeros((N_CORES, T * P), np.int64)
    bidx = np.zeros((N_CORES, T * P), np.int64)
    labmap = np.full((N_CORES, T * P), -1, np.int64)
    for c in range(N_CORES):
        pos = 0
        for k in range(4):
            la, lb, lab = buckets_per_core[c][k]
            n = len(la)
            cap = tcnt[k] * P
            aidx[c, pos:pos + n] = la - (HALF if k >= 2 else 0)
            bidx[c, pos:pos + n] = lb - (HALF if k % 2 else 0)
            labmap[c, pos:pos + n] = lab
            pos += cap
    la_s = np.stack([_wrap16(aidx[c]) for c in range(N_CORES)])
    lb_s = np.stack([_wrap16(bidx[c]) for c in range(N_CORES)])
    return dict(la=la_s, lb=lb_s, tcnt=tcnt, T=T, labmap=labmap)


# ------------------------------------------------------------- device kernel

def build_bass(n_nodes, bpc, groups, sli, shi, nsl, nsh, tcnt,
               in_c, hid_c, out_c, bias_zero=True):
    from concourse import bacc, bass, mybir
    import concourse.tile as tile

    NPAD = N_CORES * bpc * P
    NROWS = HEAD + NPAD + P
    CNT = CAPL + CAPH
    NS = nsl + nsh
    T = int(sum(tcnt))
    NAG = bpc // GPA  # allgather groups per core
    f32 = mybir.dt.float32
    bf16 = mybir.dt.bfloat16
    EIDX_N = bpc * CNT * P

    nc = bacc.Bacc("TRN2", target_bir_lowering=False, debug=False,
                   num_devices=N_CORES, num_swdge_queues=4)

    # inputs
    t0_d = nc.dram_tensor("xt", [NROWS, in_c], bf16, kind="ExternalInput")
    w_d = [nc.dram_tensor(f"W{i+1}", s, bf16, kind="ExternalInput")
           for i, s in enumerate([[in_c, hid_c], [hid_c, hid_c], [hid_c, out_c]])]
    b_d = [nc.dram_tensor(f"b{i+1}", [s], f32, kind="ExternalInput")
           for i, s in enumerate([hid_c, hid_c, out_c])]
    eidx_d = nc.dram_tensor("eidx", [P, EIDX_N // 16], mybir.dt.int16,
                            kind="ExternalInput")
    spidx_d = nc.dram_tensor("spidx", [P, max(NS * P // 16, 16)],
                             mybir.dt.int16, kind="ExternalInput")
    sind_d = nc.dram_tensor("sind", [P, max(NS, 1) * P], bf16,
                            kind="ExternalInput")
    dinv1_d = nc.dram_tensor("dinv1", [P, bpc], f32, kind="ExternalInput")
    dinv2_d = nc.dram_tensor("dinv2", [P, bpc], f32, kind="ExternalInput")
    la_d = nc.dram_tensor("la", [P, T * P // 16], mybir.dt.int16,
                          kind="ExternalInput")
    lb_d = nc.dram_tensor("lb", [P, T * P // 16], mybir.dt.int16,
                          kind="ExternalInput")
    out_d = nc.dram_tensor("out", [P, T], f32, kind="ExternalOutput")

    # internal DRAM
    t_d = [t0_d] + [nc.dram_tensor(f"T{l}", [NROWS, in_c], bf16,
                                   kind="Internal", addr_space="Shared")
                    for l in (1, 2)]
    zs_d = [nc.dram_tensor(f"zs{l}", [bpc * P, hid_c], bf16, kind="Internal")
            for l in range(2)]
    zs3_d = nc.dram_tensor("zs3", [bpc * P, out_c], f32, kind="Internal")
    z3_d = nc.dram_tensor("z3f", [NPAD, out_c], f32, kind="Internal",
                          addr_space="Shared")

    gq = [0]

    def next_q():
        q = gq[0]
        gq[0] = (q + 1) % 4
        return q

    lo_off = np.concatenate([[0], np.cumsum(sli)]).astype(int)
    hi_off = np.concatenate([[0], np.cumsum(shi)]).astype(int)

    with tile.TileContext(nc) as tc:
        with (
            tc.tile_pool(name="consts", bufs=1) as cst,
            tc.tile_pool(name="gath", bufs=4) as gp,
            tc.tile_pool(name="spill", bufs=1) as sp,
            tc.tile_pool(name="work", bufs=6) as wp,
            tc.tile_pool(name="zgp", bufs=2) as zp,
            tc.tile_pool(name="dec", bufs=2) as dp,
            tc.tile_pool(name="psum", bufs=4, space="PSUM") as ps,
            tc.tile_pool(name="psum2", bufs=2, space="PSUM") as ps2,
        ):
            # ---- constants
            ident = cst.tile([P, P], bf16)
            nc.gpsimd.memset(ident[:], 0.0)
            nc.gpsimd.affine_select(
                out=ident[:], in_=ident[:],
                compare_op=mybir.AluOpType.not_equal, fill=1.0,
                base=0, pattern=[[-1, P]], channel_multiplier=1)
            identf = cst.tile([P, P], f32)
            nc.gpsimd.memset(identf[:], 0.0)
            nc.gpsimd.affine_select(
                out=identf[:], in_=identf[:],
                compare_op=mybir.AluOpType.not_equal, fill=1.0,
                base=0, pattern=[[-1, P]], channel_multiplier=1)

            zero_sb = cst.tile([P, in_c], bf16)
            nc.vector.memset(zero_sb[:], 0.0)

            eidx_sb = cst.tile([P, EIDX_N // 16], mybir.dt.int16)
            nc.sync.dma_start(eidx_sb[:], eidx_d[:, :])
            if NS:
                spidx_sb = cst.tile([P, NS * P // 16], mybir.dt.int16)
                nc.sync.dma_start(spidx_sb[:], spidx_d[:, :NS * P // 16])
                sind_sb = cst.tile([P, NS * P], bf16)
                nc.sync.dma_start(sind_sb[:], sind_d[:, :NS * P])
            la_sb = cst.tile([P, T * P // 16], mybir.dt.int16)
            lb_sb = cst.tile([P, T * P // 16], mybir.dt.int16)
            nc.sync.dma_start(la_sb[:], la_d[:, :])
            nc.sync.dma_start(lb_sb[:], lb_d[:, :])
            dinv1_sb = cst.tile([P, bpc], f32)
            dinv2_sb = cst.tile([P, bpc], f32)
            nc.sync.dma_start(dinv1_sb[:], dinv1_d[:, :])
            nc.sync.dma_start(dinv2_sb[:], dinv2_d[:, :])

            w_sb = []
            bfull_sb = []
            if not bias_zero:
                ones_row = cst.tile([1, P], bf16)
                nc.vector.memset(ones_row[:], 1.0)
            for l in range(3):
                oc_l = out_c if l == 2 else hid_c
                wt = cst.tile([hid_c if l else in_c, oc_l], bf16)
                nc.sync.dma_start(wt[:], w_d[l][:, :])
                w_sb.append(wt)
                if not bias_zero:
                    bt = cst.tile([1, oc_l], f32)
                    nc.sync.dma_start(bt[:], b_d[l][None, :])
                    btb = cst.tile([1, oc_l], bf16)
                    nc.vector.tensor_copy(out=btb[:], in_=bt[:])
                    b_ps = ps2.tile([P, oc_l], f32, tag="bps", space="PSUM")
                    nc.tensor.matmul(out=b_ps[:], lhsT=ones_row[:], rhs=btb[:],
                                     start=True, stop=True)
                    bft = cst.tile([P, oc_l], f32)
                    nc.vector.tensor_copy(out=bft[:], in_=b_ps[:])
                    bfull_sb.append(bft)

            # zero head/tail rows of the z tables (T0 comes pre-zeroed)
            for l in (1, 2):
                nc.sync.dma_start(t_d[l][0:HEAD, :], zero_sb[:])
                nc.sync.dma_start(t_d[l][HEAD + NPAD:NROWS, :], zero_sb[:])

            # ---- 3 GCN layers
            for l in range(3):
                oc = out_c if l == 2 else hid_c
                tab = t_d[l]

                if NS:
                    spt = sp.tile([P, NS * in_c], bf16, tag="sp")
                    sp3 = spt[:].rearrange("p (c f) -> p c f", c=NS)
                    if nsl:
                        nc.gpsimd.dma_gather(
                            out_ap=sp3[:, 0:nsl, :] if nsh else sp3,
                            in_ap=tab[:, :],
                            idxs_ap=spidx_sb[:, 0:nsl * 8],
                            num_idxs=nsl * P, num_idxs_reg=nsl * P,
                            elem_size=in_c, single_packet=False,
                            queue_num=next_q())
                    if nsh:
                        nc.gpsimd.dma_gather(
                            out_ap=sp3[:, nsl:, :] if nsl else sp3,
                            in_ap=tab[HALF:, :],
                            idxs_ap=spidx_sb[:, nsl * 8:NS * 8],
                            num_idxs=nsh * P, num_idxs_reg=nsh * P,
                            elem_size=in_c, single_packet=False,
                            queue_num=next_q())

                stage = zs_d[l] if l < 2 else zs3_d
                zg = None
                goff = 0
                b0 = 0
                for gs in groups:
                    gt = gp.tile([P, GRP * CNT * in_c], bf16, tag="gt")
                    g3 = gt[:].rearrange("p (c f) -> p c f", c=GRP * CNT)
                    lo_n = gs * CAPL * P
                    hi_n = gs * CAPH * P
                    nc.gpsimd.dma_gather(
                        out_ap=g3[:, 0:gs * CAPL, :],
                        in_ap=tab[:, :],
                        idxs_ap=eidx_sb[:, goff // 16:(goff + lo_n) // 16],
                        num_idxs=lo_n, num_idxs_reg=lo_n,
                        elem_size=in_c, single_packet=False,
                        queue_num=next_q())
                    nc.gpsimd.dma_gather(
                        out_ap=g3[:, gs * CAPL:gs * CNT, :],
                        in_ap=tab[HALF:, :],
                        idxs_ap=eidx_sb[:, (goff + lo_n) // 16:
                                        (goff + lo_n + hi_n) // 16],
                        num_idxs=hi_n, num_idxs_reg=hi_n,
                        elem_size=in_c, single_packet=False,
                        queue_num=next_q())
                    goff += lo_n + hi_n

                    for i in range(gs):
                        b = b0 + i
                        j = b % GPA  # position within the allgather group
                        if j == 0:
                            zg = zp.tile([P, GPA * oc],
                                         bf16 if l < 2 else f32, tag="zg")
                        chunks = [i * CAPL + k for k in range(CAPL)] + \
                                 [gs * CAPL + i * CAPH + k for k in range(CAPH)]
                        n_sp = int(sli[b] + shi[b])

                        agg_ps = ps.tile([P, 4 * P], f32, tag="agg", space="PSUM")
                        mms = []
                        for w in range((CNT + 3) // 4):
                            cs = chunks[w * 4:(w + 1) * 4]
                            runs = []
                            run = [cs[0]]
                            for cc in cs[1:]:
                                if cc == run[-1] + 1:
                                    run.append(cc)
                                else:
                                    runs.append(run)
                                    run = [cc]
                            runs.append(run)
                            col0 = 0
                            for run in runs:
                                mms.append(("id", col0, len(run), run[0]))
                                col0 += len(run)
                        for kk in range(n_sp):
                            if kk < sli[b]:
                                sc = int(lo_off[b] + kk)
                            else:
                                sc = int(nsl + hi_off[b] + (kk - sli[b]))
                            mms.append(("sp", 0, 1, sc))

                        for mi, (kind, col0, width, src0) in enumerate(mms):
                            last = mi == len(mms) - 1
                            if kind == "id":
                                nc.tensor.matmul(
                                    out=agg_ps[:, col0 * P:(col0 + width) * P],
                                    lhsT=ident[:],
                                    rhs=gt[:, src0 * P:(src0 + width) * P],
                                    start=(mi == 0), stop=last,
                                    skip_group_check=True)
                            else:
                                nc.tensor.matmul(
                                    out=agg_ps[:, 0:P],
                                    lhsT=sind_sb[:, src0 * P:(src0 + 1) * P],
                                    rhs=sp3[:, src0, :],
                                    start=False, stop=last,
                                    skip_group_check=True)

                        agg_sb = wp.tile([P, P], f32, tag="agg_sb")
                        nc.vector.tensor_reduce(
                            out=agg_sb[:],
                            in_=agg_ps[:].rearrange("p (a f) -> p f a", a=4),
                            axis=mybir.AxisListType.X, op=mybir.AluOpType.add)

                        t_ps = ps2.tile([P, P], f32, tag="tp", space="PSUM")
                        nc.tensor.matmul(out=t_ps[:], lhsT=agg_sb[:],
                                         rhs=identf[:], start=True, stop=True)
                        aggt = wp.tile([P, P], bf16, tag="aggt")
                        nc.vector.tensor_copy(out=aggt[:], in_=t_ps[:])

                        z_ps = ps2.tile([P, oc], f32, tag="z", space="PSUM")
                        nc.tensor.matmul(out=z_ps[:], lhsT=aggt[:],
                                         rhs=w_sb[l][:], start=True, stop=True)
                        if bias_zero:
                            nc.scalar.activation(
                                out=zg[:, j * oc:(j + 1) * oc], in_=z_ps[:],
                                func=(mybir.ActivationFunctionType.Relu if l < 2
                                      else mybir.ActivationFunctionType.Copy),
                                scale=(dinv2_sb if l < 2 else dinv1_sb)[:, b:b + 1])
                        else:
                            sc_ap = (dinv2_sb if l < 2 else dinv1_sb)[:, b:b + 1]
                            if l < 2:
                                tmp = wp.tile([P, oc], f32, tag="zb")
                                nc.vector.scalar_tensor_tensor(
                                    out=tmp[:], in0=z_ps[:], scalar=sc_ap,
                                    in1=bfull_sb[l][:],
                                    op0=mybir.AluOpType.mult,
                                    op1=mybir.AluOpType.add)
                                nc.scalar.activation(
                                    out=zg[:, j * oc:(j + 1) * oc], in_=tmp[:],
                                    func=mybir.ActivationFunctionType.Relu)
                            else:
                                nc.vector.scalar_tensor_tensor(
                                    out=zg[:, j * oc:(j + 1) * oc],
                                    in0=z_ps[:], scalar=sc_ap,
                                    in1=bfull_sb[l][:],
                                    op0=mybir.AluOpType.mult,
                                    op1=mybir.AluOpType.add)

                        if j == GPA - 1:
                            # stage this allgather group's z and replicate it
                            ag = b // GPA
                            r0 = ag * GPA * P
                            nc.sync.dma_start(
                                stage[r0:r0 + GPA * P, :].rearrange(
                                    "(g p) f -> p g f", g=GPA),
                                zg[:].rearrange("p (g f) -> p g f", g=GPA))
                            if l < 2:
                                o0 = HEAD + ag * (N_CORES * GPA * P)
                                nc.gpsimd.collective_compute(
                                    "AllGather", mybir.AluOpType.bypass,
                                    replica_groups=[list(range(N_CORES))],
                                    ins=[stage[r0:r0 + GPA * P, :]],
                                    outs=[t_d[l + 1][o0:o0 + N_CORES * GPA * P, :]])
                            else:
                                o0 = ag * (N_CORES * GPA * P)
                                nc.gpsimd.collective_compute(
                                    "AllGather", mybir.AluOpType.bypass,
                                    replica_groups=[list(range(N_CORES))],
                                    ins=[stage[r0:r0 + GPA * P, :]],
                                    outs=[z3_d[o0:o0 + N_CORES * GPA * P, :]])
                    b0 += gs

            # ---- decode
            res = cst.tile([P, T], f32)
            tbase = 0
            for k in range(4):
                tk = int(tcnt[k])
                if tk == 0:
                    continue
                a_tab = z3_d[HALF:, :] if k >= 2 else z3_d[:, :]
                b_tab = z3_d[HALF:, :] if k % 2 else z3_d[:, :]
                for s0 in range(0, tk, DEC_SC):
                    scn = min(DEC_SC, tk - s0)
                    toff = tbase + s0
                    ga = dp.tile([P, DEC_SC * out_c], f32, tag="ga")
                    gb = dp.tile([P, DEC_SC * out_c], f32, tag="gb")
                    nc.gpsimd.dma_gather(
                        out_ap=ga[:, :scn * out_c].rearrange(
                            "p (c f) -> p c f", c=scn),
                        in_ap=a_tab,
                        idxs_ap=la_sb[:, toff * 8:(toff + scn) * 8],
                        num_idxs=scn * P, num_idxs_reg=scn * P,
                        elem_size=out_c, single_packet=False,
                        queue_num=next_q())
                    nc.gpsimd.dma_gather(
                        out_ap=gb[:, :scn * out_c].rearrange(
                            "p (c f) -> p c f", c=scn),
                        in_ap=b_tab,
                        idxs_ap=lb_sb[:, toff * 8:(toff + scn) * 8],
                        num_idxs=scn * P, num_idxs_reg=scn * P,
                        elem_size=out_c, single_packet=False,
                        queue_num=next_q())
                    nc.vector.tensor_tensor(
                        out=ga[:, :scn * out_c], in0=ga[:, :scn * out_c],
                        in1=gb[:, :scn * out_c], op=mybir.AluOpType.mult)
                    nc.vector.tensor_reduce(
                        out=res[:, toff:toff + scn],
                        in_=ga[:, :scn * out_c].rearrange(
                            "p (c f) -> p c f", c=scn),
                        axis=mybir.AxisListType.X, op=mybir.AluOpType.add)
                tbase += tk
            nc.sync.dma_start(out_d[:, :], res[:])

    nc.finalize()
    return nc


# ---------------------------------------------------------------- entry point

def kernel(x, W1, b1, W2, b2, W3, b3, edge_index, edge_label_index):
    from concourse.bass_utils import run_bass_kernel_spmd

    x = np.ascontiguousarray(np.asarray(x, dtype=np.float32))
    n_nodes, in_c = x.shape
    hid_c = np.asarray(W2).shape[0]
    out_c = np.asarray(W3).shape[1]
    n_label = np.asarray(edge_label_index).shape[1]
    bpc = int(np.ceil(n_nodes / (N_CORES * P)))
    npad = N_CORES * bpc * P
    nrows = HEAD + npad + P

    ed = prepare_edges(edge_index, n_nodes, bpc)
    lb = prepare_labels(edge_label_index, n_label, ed["rowmap"])

    bias_zero = all(np.all(np.asarray(b) == 0) for b in (b1, b2, b3))
    nc = build_bass(n_nodes, bpc, ed["groups"], ed["sli"], ed["shi"],
                    ed["nsl"], ed["nsh"], lb["tcnt"], in_c, hid_c, out_c,
                    bias_zero=bias_zero)

    # host-side layer-0 table: x * dinv, bf16, group-major rows, zero pads
    xp = np.zeros((npad, in_c), np.float32)
    xp[:n_nodes] = x
    xt = np.zeros((nrows, in_c), ml_dtypes.bfloat16)
    xt[ed["rowmap"]] = (xp * ed["dinv"][:, None]).astype(ml_dtypes.bfloat16)

    common = {
        "xt": xt,
        "W1": np.asarray(W1, np.float32).astype(ml_dtypes.bfloat16),
        "W2": np.asarray(W2, np.float32).astype(ml_dtypes.bfloat16),
        "W3": np.asarray(W3, np.float32).astype(ml_dtypes.bfloat16),
        "b1": np.ascontiguousarray(np.asarray(b1, np.float32)),
        "b2": np.ascontiguousarray(np.asarray(b2, np.float32)),
        "b3": np.ascontiguousarray(np.asarray(b3, np.float32)),
    }
    in_maps = []
    for c in range(N_CORES):
        m = dict(common)
        m["eidx"] = np.ascontiguousarray(ed["eidx"][c])
        m["spidx"] = np.ascontiguousarray(
            ed["spidx"][c] if ed["nsl"] + ed["nsh"] else
            np.zeros((P, 16), np.int16))
        m["sind"] = np.ascontiguousarray(ed["sind"][c])
        m["dinv1"] = np.ascontiguousarray(ed["dinv1"][c])
        m["dinv2"] = np.ascontiguousarray(ed["dinv2"][c])
        m["la"] = np.ascontiguousarray(lb["la"][c])
        m["lb"] = np.ascontiguousarray(lb["lb"][c])
        in_maps.append(m)

    res = run_bass_kernel_spmd(nc, in_maps, core_ids=list(range(N_CORES)))

    out = np.zeros((n_label,), np.float32)
    for c in range(N_CORES):
        o = res.results[c]["out"]  # [P, T]
        flat = o.T.reshape(-1)
        lm = lb["labmap"][c]
        valid = lm >= 0
        out[lm[valid]] = flat[valid]
    return out


# revision 29
# speedup vs baseline: 1.4040x; 1.0498x over previous
"""GCN message-passing kernel for Trainium2, 8 NeuronCores (v4).

Math (reference): 3-layer GCN with symmetric normalization and self-loops,
then dot-product decode over label edge pairs.

Reformulations:
  - A_hat @ (x @ W) == (A_hat @ x) @ W: aggregate, then dense matmul.
  - Degree norm is separable: dinv[src] folds into the gather table
    (rows hold z*dinv), dinv[dst] folds into the output activation
    (relu(c*x) == c*relu(x), c>0). No per-edge weights remain.
  - Aggregation: per dst block, edges are slot-major (chunk k holds the
    k-th in-edge of each dst; slot == dst_local). dma_gather lands
    [slot, chunk, feat] tiles; the Tensor engine sums chunks into a
    4-wide PSUM tile via identity matmuls (512-col windows), DVE folds
    4->1, PE transposes via an identity rhs, then z matmul + ACT
    (scale=dinv^2 + relu + bf16 cast). Per-dst overflow beyond the caps
    goes through spill chunks with host-prebuilt one-hot indicators.
  - bf16 tables with zero head/tail rows so pad slots gather exact zeros.
  - Table rows are GROUP-MAJOR: row(n) = HEAD + g*8*GPA*128 + c*GPA*128
    + j*128 + slot, so one small AllGather per block-group g replicates
    z while later groups still compute (no end-of-layer mega-collective).
    The layer-0 table (x*dinv bf16) is built on the host and fed as an
    input directly -- no startup collective at all.

Decode: labels bucketed by (a<32768, b<32768) on remapped rows, z3 fp32
[N,64]; gather both sides, DVE multiply+reduce, host inverse-permutes.
"""

import numpy as np
import ml_dtypes

P = 128
N_CORES = 8
HEAD = 128          # zero rows at table head
HALF = 32768
CAPL = 13           # slot-major chunk cap, low-side (table row < HALF)
CAPH = 7            # high-side cap
GRP = 3             # dst blocks per gather group
GPA = 49            # dst blocks per allgather group (divides bpc)
DEC_SC = 16         # decode sub-call size in 128-label chunks


def _wrap16(flat_idx):
    t = flat_idx.astype(np.int16).reshape(-1, 16).T  # [16, n/16]
    return np.tile(t, (8, 1))  # [128, n/16]


def _row_map(n, bpc):
    """table row of node n under the group-major layout (HEAD included)."""
    c = n // (bpc * P)
    ln = n % (bpc * P)
    i = ln // P
    g = i // GPA
    j = i % GPA
    return HEAD + g * (N_CORES * GPA * P) + c * (GPA * P) + j * P + (n % P)


# ---------------------------------------------------------------- host prep

def prepare_edges(edge_index, n_nodes, bpc):
    npad = N_CORES * bpc * P
    padhi = HEAD + npad - HALF  # zero-tail row, relative to the high view

    src = np.asarray(edge_index[0], dtype=np.int64)
    dst = np.asarray(edge_index[1], dtype=np.int64)
    loops = np.arange(n_nodes, dtype=np.int64)
    esrc = np.concatenate([src, loops])
    edst = np.concatenate([dst, loops])

    deg = np.bincount(edst, minlength=npad).astype(np.float64)
    dinv = np.where(deg > 0, 1.0 / np.sqrt(np.maximum(deg, 1.0)), 0.0)

    rowmap = _row_map(np.arange(npad, dtype=np.int64), bpc)
    erow = rowmap[esrc]
    side = (erow >= HALF).astype(np.int64)  # 0 low, 1 high
    key = edst * 2 + side
    order = np.argsort(key, kind="stable")
    sk = key[order]
    srow = erow[order]
    sdst = edst[order]
    starts = np.searchsorted(sk, np.arange(2 * npad))
    rank = np.arange(sk.size) - starts[sk]
    sside = sk & 1

    lowmat = np.zeros((npad, CAPL), np.int16)           # pad idx 0 -> zero row
    highmat = np.full((npad, CAPH), padhi, np.int16)    # pad idx -> zero tail
    sel = (sside == 0) & (rank < CAPL)
    lowmat[sdst[sel], rank[sel]] = srow[sel].astype(np.int16)
    sel = (sside == 1) & (rank < CAPH)
    highmat[sdst[sel], rank[sel]] = (srow[sel] - HALF).astype(np.int16)

    # spill edges per (core, block, side)
    cap_arr = np.where(sside == 0, CAPL, CAPH)
    sp = rank >= cap_arr
    sp_dst, sp_row, sp_side = sdst[sp], srow[sp], sside[sp]
    sp_blk = sp_dst >> 7
    sp_core = sp_blk // bpc
    sp_bi = sp_blk % bpc
    cnt = np.zeros((N_CORES, bpc, 2), np.int64)
    np.add.at(cnt, (sp_core, sp_bi, sp_side), 1)
    sli = np.ceil(cnt[:, :, 0] / P).astype(np.int64).max(axis=0)  # [bpc]
    shi = np.ceil(cnt[:, :, 1] / P).astype(np.int64).max(axis=0)
    nsl, nsh = int(sli.sum()), int(shi.sum())
    NS = nsl + nsh

    sp_idx = np.zeros((N_CORES, max(NS, 1) * P), np.int16)
    sp_idx[:, nsl * P:] = padhi
    sind = np.zeros((N_CORES, P, max(NS, 1) * P), ml_dtypes.bfloat16)
    lo_off = np.concatenate([[0], np.cumsum(sli)])
    hi_off = np.concatenate([[0], np.cumsum(shi)])
    ordsp = np.lexsort((sp_dst, sp_bi, sp_core))
    sp_dst, sp_row, sp_side = sp_dst[ordsp], sp_row[ordsp], sp_side[ordsp]
    sp_core, sp_bi = sp_core[ordsp], sp_bi[ordsp]
    for c in range(N_CORES):
        m = sp_core == c
        d, r, sd, bi = sp_dst[m], sp_row[m], sp_side[m], sp_bi[m]
        for b in range(bpc):
            mb = bi == b
            dl = (d[mb] & 127).astype(np.int64)
            rowb, sdb = r[mb], sd[mb]
            lo = sdb == 0
            nl = int(lo.sum())
            pos = int(lo_off[b]) * P
            sp_idx[c, pos:pos + nl] = rowb[lo].astype(np.int16)
            jj = pos + np.arange(nl)
            sind[c][jj % P, (jj // P) * P + dl[lo]] = 1.0
            nh = int((~lo).sum())
            pos = (nsl + int(hi_off[b])) * P
            sp_idx[c, pos:pos + nh] = (rowb[~lo] - HALF).astype(np.int16)
            jj = pos + np.arange(nh)
            sind[c][jj % P, (jj // P) * P + dl[~lo]] = 1.0

    # main idx stream per core: per gather group (GRP blocks), low then high
    groups = [GRP] * (bpc // GRP) + ([bpc % GRP] if bpc % GRP else [])
    eidx = []
    for c in range(N_CORES):
        parts = []
        b0 = 0
        for gs in groups:
            rows = np.arange((c * bpc + b0) * P, (c * bpc + b0 + gs) * P)
            lm = lowmat[rows].reshape(gs, P, CAPL).transpose(0, 2, 1)
            parts.append(lm.ravel())
            hm = highmat[rows].reshape(gs, P, CAPH).transpose(0, 2, 1)
            parts.append(hm.ravel())
            b0 += gs
        eidx.append(_wrap16(np.concatenate(parts)))
    eidx = np.stack(eidx)

    spidx = np.stack([_wrap16(sp_idx[c]) for c in range(N_CORES)])

    dpc = dinv.reshape(N_CORES, bpc, P)
    dinv1 = np.ascontiguousarray(dpc.transpose(0, 2, 1)).astype(np.float32)
    dinv2 = (dinv1 ** 2).astype(np.float32)

    return dict(eidx=eidx, spidx=spidx, sind=sind, sli=sli, shi=shi,
                nsl=nsl, nsh=nsh, dinv=dinv, dinv1=dinv1, dinv2=dinv2,
                rowmap=rowmap, groups=groups)


def prepare_labels(edge_label_index, n_label, rowmap):
    """Bucket labels by (rowa<HALF, rowb<HALF) per core, pad to 128s.

    Uses the remapped z3 rows (rowmap - HEAD)."""
    a = rowmap[np.asarray(edge_label_index[0], dtype=np.int64)] - HEAD
    b = rowmap[np.asarray(edge_label_index[1], dtype=np.int64)] - HEAD
    per = n_label // N_CORES
    buckets_per_core = []
    for c in range(N_CORES):
        la = a[c * per:(c + 1) * per]
        lb = b[c * per:(c + 1) * per]
        lab = np.arange(c * per, (c + 1) * per)
        bid = (la >= HALF) * 2 + (lb >= HALF)
        buckets_per_core.append([(la[bid == k], lb[bid == k], lab[bid == k])
                                 for k in range(4)])
    tcnt = [max(int(np.ceil(len(buckets_per_core[c][k][0]) / P))
                for c in range(N_CORES)) for k in range(4)]
    T = sum(tcnt)
    aidx = np.zeros((N_CORES, T * P), np.int64)
    bidx = np.zeros((N_CORES, T * P), np.int64)
    labmap = np.full((N_CORES, T * P), -1, np.int64)
    for c in range(N_CORES):
        pos = 0
        for k in range(4):
            la, lb, lab = buckets_per_core[c][k]
            n = len(la)
            cap = tcnt[k] * P
            aidx[c, pos:pos + n] = la - (HALF if k >= 2 else 0)
            bidx[c, pos:pos + n] = lb - (HALF if k % 2 else 0)
            labmap[c, pos:pos + n] = lab
            pos += cap
    la_s = np.stack([_wrap16(aidx[c]) for c in range(N_CORES)])
    lb_s = np.stack([_wrap16(bidx[c]) for c in range(N_CORES)])
    return dict(la=la_s, lb=lb_s, tcnt=tcnt, T=T, labmap=labmap)


# ------------------------------------------------------------- device kernel

def build_bass(n_nodes, bpc, groups, sli, shi, nsl, nsh, tcnt,
               in_c, hid_c, out_c, bias_zero=True):
    from concourse import bacc, bass, mybir
    import concourse.tile as tile

    NPAD = N_CORES * bpc * P
    NROWS = HEAD + NPAD + P
    CNT = CAPL + CAPH
    NS = nsl + nsh
    T = int(sum(tcnt))
    NAG = bpc // GPA  # allgather groups per core
    f32 = mybir.dt.float32
    bf16 = mybir.dt.bfloat16
    EIDX_N = bpc * CNT * P

    nc = bacc.Bacc("TRN2", target_bir_lowering=False, debug=False,
                   num_devices=N_CORES, num_swdge_queues=4)

    # inputs
    t0_d = nc.dram_tensor("xt", [NROWS, in_c], bf16, kind="ExternalInput")
    w_d = [nc.dram_tensor(f"W{i+1}", s, bf16, kind="ExternalInput")
           for i, s in enumerate([[in_c, hid_c], [hid_c, hid_c], [hid_c, out_c]])]
    b_d = [nc.dram_tensor(f"b{i+1}", [s], f32, kind="ExternalInput")
           for i, s in enumerate([hid_c, hid_c, out_c])]
    eidx_d = nc.dram_tensor("eidx", [P, EIDX_N // 16], mybir.dt.int16,
                            kind="ExternalInput")
    spidx_d = nc.dram_tensor("spidx", [P, max(NS * P // 16, 16)],
                             mybir.dt.int16, kind="ExternalInput")
    sind_d = nc.dram_tensor("sind", [P, max(NS, 1) * P], bf16,
                            kind="ExternalInput")
    dinv1_d = nc.dram_tensor("dinv1", [P, bpc], f32, kind="ExternalInput")
    dinv2_d = nc.dram_tensor("dinv2", [P, bpc], f32, kind="ExternalInput")
    la_d = nc.dram_tensor("la", [P, T * P // 16], mybir.dt.int16,
                          kind="ExternalInput")
    lb_d = nc.dram_tensor("lb", [P, T * P // 16], mybir.dt.int16,
                          kind="ExternalInput")
    out_d = nc.dram_tensor("out", [P, T], f32, kind="ExternalOutput")

    # internal DRAM
    t_d = [t0_d] + [nc.dram_tensor(f"T{l}", [NROWS, in_c], bf16,
                                   kind="Internal", addr_space="Shared")
                    for l in (1, 2)]
    zs_d = [nc.dram_tensor(f"zs{l}", [bpc * P, hid_c], bf16, kind="Internal")
            for l in range(2)]
    zs3_d = nc.dram_tensor("zs3", [bpc * P, out_c], f32, kind="Internal")
    z3_d = nc.dram_tensor("z3f", [NPAD, out_c], f32, kind="Internal",
                          addr_space="Shared")

    gq = [0]

    def next_q():
        q = gq[0]
        gq[0] = (q + 1) % 4
        return q

    lo_off = np.concatenate([[0], np.cumsum(sli)]).astype(int)
    hi_off = np.concatenate([[0], np.cumsum(shi)]).astype(int)

    with tile.TileContext(nc) as tc:
        with (
            tc.tile_pool(name="consts", bufs=1) as cst,
            tc.tile_pool(name="gath", bufs=4) as gp,
            tc.tile_pool(name="spill", bufs=1) as sp,
            tc.tile_pool(name="work", bufs=6) as wp,
            tc.tile_pool(name="zgp", bufs=2) as zp,
            tc.tile_pool(name="dec", bufs=2) as dp,
            tc.tile_pool(name="psum", bufs=3, space="PSUM") as ps,
            tc.tile_pool(name="psum2", bufs=3, space="PSUM") as ps2,
            tc.tile_pool(name="psum3", bufs=2, space="PSUM") as ps3,
        ):
            # split any gather into <=16-chunk (2048 idx) subcalls on
            # rotating queues so the Pool engine never FIFO-stalls and all
            # four SWDGE queues drain concurrently
            def emit_gather(out3, tab_ap, idx_tile, idx_chunk0, ochunk0, nch,
                            elem):
                s0 = 0
                while s0 < nch:
                    sn = min(16, nch - s0)
                    nc.gpsimd.dma_gather(
                        out_ap=out3[:, ochunk0 + s0:ochunk0 + s0 + sn, :],
                        in_ap=tab_ap,
                        idxs_ap=idx_tile[:, (idx_chunk0 + s0) * 8:
                                         (idx_chunk0 + s0 + sn) * 8],
                        num_idxs=sn * P, num_idxs_reg=sn * P,
                        elem_size=elem, single_packet=False,
                        queue_num=next_q())
                    s0 += sn
            # ---- constants
            ident = cst.tile([P, P], bf16)
            nc.gpsimd.memset(ident[:], 0.0)
            nc.gpsimd.affine_select(
                out=ident[:], in_=ident[:],
                compare_op=mybir.AluOpType.not_equal, fill=1.0,
                base=0, pattern=[[-1, P]], channel_multiplier=1)
            identf = cst.tile([P, P], f32)
            nc.gpsimd.memset(identf[:], 0.0)
            nc.gpsimd.affine_select(
                out=identf[:], in_=identf[:],
                compare_op=mybir.AluOpType.not_equal, fill=1.0,
                base=0, pattern=[[-1, P]], channel_multiplier=1)

            zero_sb = cst.tile([P, in_c], bf16)
            nc.vector.memset(zero_sb[:], 0.0)

            eidx_sb = cst.tile([P, EIDX_N // 16], mybir.dt.int16)
            nc.sync.dma_start(eidx_sb[:], eidx_d[:, :])
            if NS:
                spidx_sb = cst.tile([P, NS * P // 16], mybir.dt.int16)
                nc.sync.dma_start(spidx_sb[:], spidx_d[:, :NS * P // 16])
                sind_sb = cst.tile([P, NS * P], bf16)
                nc.sync.dma_start(sind_sb[:], sind_d[:, :NS * P])
            la_sb = cst.tile([P, T * P // 16], mybir.dt.int16)
            lb_sb = cst.tile([P, T * P // 16], mybir.dt.int16)
            nc.sync.dma_start(la_sb[:], la_d[:, :])
            nc.sync.dma_start(lb_sb[:], lb_d[:, :])
            dinv1_sb = cst.tile([P, bpc], f32)
            dinv2_sb = cst.tile([P, bpc], f32)
            nc.sync.dma_start(dinv1_sb[:], dinv1_d[:, :])
            nc.sync.dma_start(dinv2_sb[:], dinv2_d[:, :])

            w_sb = []
            bfull_sb = []
            if not bias_zero:
                ones_row = cst.tile([1, P], bf16)
                nc.vector.memset(ones_row[:], 1.0)
            for l in range(3):
                oc_l = out_c if l == 2 else hid_c
                wt = cst.tile([hid_c if l else in_c, oc_l], bf16)
                nc.sync.dma_start(wt[:], w_d[l][:, :])
                w_sb.append(wt)
                if not bias_zero:
                    bt = cst.tile([1, oc_l], f32)
                    nc.sync.dma_start(bt[:], b_d[l][None, :])
                    btb = cst.tile([1, oc_l], bf16)
                    nc.vector.tensor_copy(out=btb[:], in_=bt[:])
                    b_ps = ps3.tile([P, oc_l], f32, tag="bps", space="PSUM")
                    nc.tensor.matmul(out=b_ps[:], lhsT=ones_row[:], rhs=btb[:],
                                     start=True, stop=True)
                    bft = cst.tile([P, oc_l], f32)
                    nc.vector.tensor_copy(out=bft[:], in_=b_ps[:])
                    bfull_sb.append(bft)

            # zero head/tail rows of the z tables (T0 comes pre-zeroed)
            for l in (1, 2):
                nc.sync.dma_start(t_d[l][0:HEAD, :], zero_sb[:])
                nc.sync.dma_start(t_d[l][HEAD + NPAD:NROWS, :], zero_sb[:])

            # ---- 3 GCN layers
            for l in range(3):
                oc = out_c if l == 2 else hid_c
                tab = t_d[l]

                if NS:
                    spt = sp.tile([P, NS * in_c], bf16, tag="sp")
                    sp3 = spt[:].rearrange("p (c f) -> p c f", c=NS)
                    if nsl:
                        emit_gather(sp3, tab[:, :], spidx_sb, 0, 0, nsl, in_c)
                    if nsh:
                        emit_gather(sp3, tab[HALF:, :], spidx_sb, nsl, nsl,
                                    nsh, in_c)

                stage = zs_d[l] if l < 2 else zs3_d
                zg = None
                goff = 0
                b0 = 0
                for gs in groups:
                    gt = gp.tile([P, GRP * CNT * in_c], bf16, tag="gt")
                    g3 = gt[:].rearrange("p (c f) -> p c f", c=GRP * CNT)
                    lo_n = gs * CAPL * P
                    hi_n = gs * CAPH * P
                    gch = goff // P
                    emit_gather(g3, tab[:, :], eidx_sb, gch, 0,
                                gs * CAPL, in_c)
                    emit_gather(g3, tab[HALF:, :], eidx_sb, gch + gs * CAPL,
                                gs * CAPL, gs * CAPH, in_c)
                    goff += lo_n + hi_n

                    for i in range(gs):
                        b = b0 + i
                        j = b % GPA  # position within the allgather group
                        if j == 0:
                            zg = zp.tile([P, GPA * oc],
                                         bf16 if l < 2 else f32, tag="zg")
                        chunks = [i * CAPL + k for k in range(CAPL)] + \
                                 [gs * CAPL + i * CAPH + k for k in range(CAPH)]
                        n_sp = int(sli[b] + shi[b])

                        agg_ps = ps.tile([P, 4 * P], f32, tag="agg", space="PSUM")
                        mms = []
                        for w in range((CNT + 3) // 4):
                            cs = chunks[w * 4:(w + 1) * 4]
                            runs = []
                            run = [cs[0]]
                            for cc in cs[1:]:
                                if cc == run[-1] + 1:
                                    run.append(cc)
                                else:
                                    runs.append(run)
                                    run = [cc]
                            runs.append(run)
                            col0 = 0
                            for run in runs:
                                mms.append(("id", col0, len(run), run[0]))
                                col0 += len(run)
                        for kk in range(n_sp):
                            if kk < sli[b]:
                                sc = int(lo_off[b] + kk)
                            else:
                                sc = int(nsl + hi_off[b] + (kk - sli[b]))
                            mms.append(("sp", 0, 1, sc))

                        for mi, (kind, col0, width, src0) in enumerate(mms):
                            last = mi == len(mms) - 1
                            if kind == "id":
                                nc.tensor.matmul(
                                    out=agg_ps[:, col0 * P:(col0 + width) * P],
                                    lhsT=ident[:],
                                    rhs=gt[:, src0 * P:(src0 + width) * P],
                                    start=(mi == 0), stop=last,
                                    skip_group_check=True)
                            else:
                                nc.tensor.matmul(
                                    out=agg_ps[:, 0:P],
                                    lhsT=sind_sb[:, src0 * P:(src0 + 1) * P],
                                    rhs=sp3[:, src0, :],
                                    start=False, stop=last,
                                    skip_group_check=True)

                        agg_sb = wp.tile([P, P], f32, tag="agg_sb")
                        nc.vector.tensor_reduce(
                            out=agg_sb[:],
                            in_=agg_ps[:].rearrange("p (a f) -> p f a", a=4),
                            axis=mybir.AxisListType.X, op=mybir.AluOpType.add)

                        t_ps = ps2.tile([P, P], f32, tag="tp", space="PSUM")
                        nc.tensor.matmul(out=t_ps[:], lhsT=agg_sb[:],
                                         rhs=identf[:], start=True, stop=True)
                        aggt = wp.tile([P, P], bf16, tag="aggt")
                        nc.vector.tensor_copy(out=aggt[:], in_=t_ps[:])

                        z_ps = ps3.tile([P, oc], f32, tag="z", space="PSUM")
                        nc.tensor.matmul(out=z_ps[:], lhsT=aggt[:],
                                         rhs=w_sb[l][:], start=True, stop=True)
                        if bias_zero:
                            nc.scalar.activation(
                                out=zg[:, j * oc:(j + 1) * oc], in_=z_ps[:],
                                func=(mybir.ActivationFunctionType.Relu if l < 2
                                      else mybir.ActivationFunctionType.Copy),
                                scale=(dinv2_sb if l < 2 else dinv1_sb)[:, b:b + 1])
                        else:
                            sc_ap = (dinv2_sb if l < 2 else dinv1_sb)[:, b:b + 1]
                            if l < 2:
                                tmp = wp.tile([P, oc], f32, tag="zb")
                                nc.vector.scalar_tensor_tensor(
                                    out=tmp[:], in0=z_ps[:], scalar=sc_ap,
                                    in1=bfull_sb[l][:],
                                    op0=mybir.AluOpType.mult,
                                    op1=mybir.AluOpType.add)
                                nc.scalar.activation(
                                    out=zg[:, j * oc:(j + 1) * oc], in_=tmp[:],
                                    func=mybir.ActivationFunctionType.Relu)
                            else:
                                nc.vector.scalar_tensor_tensor(
                                    out=zg[:, j * oc:(j + 1) * oc],
                                    in0=z_ps[:], scalar=sc_ap,
                                    in1=bfull_sb[l][:],
                                    op0=mybir.AluOpType.mult,
                                    op1=mybir.AluOpType.add)

                        if j == GPA - 1:
                            # stage this allgather group's z and replicate it
                            ag = b // GPA
                            r0 = ag * GPA * P
                            nc.sync.dma_start(
                                stage[r0:r0 + GPA * P, :].rearrange(
                                    "(g p) f -> p g f", g=GPA),
                                zg[:].rearrange("p (g f) -> p g f", g=GPA))
                            if l < 2:
                                o0 = HEAD + ag * (N_CORES * GPA * P)
                                nc.gpsimd.collective_compute(
                                    "AllGather", mybir.AluOpType.bypass,
                                    replica_groups=[list(range(N_CORES))],
                                    ins=[stage[r0:r0 + GPA * P, :]],
                                    outs=[t_d[l + 1][o0:o0 + N_CORES * GPA * P, :]])
                            else:
                                o0 = ag * (N_CORES * GPA * P)
                                nc.gpsimd.collective_compute(
                                    "AllGather", mybir.AluOpType.bypass,
                                    replica_groups=[list(range(N_CORES))],
                                    ins=[stage[r0:r0 + GPA * P, :]],
                                    outs=[z3_d[o0:o0 + N_CORES * GPA * P, :]])
                    b0 += gs

            # ---- decode
            res = cst.tile([P, T], f32)
            tbase = 0
            for k in range(4):
                tk = int(tcnt[k])
                if tk == 0:
                    continue
                a_tab = z3_d[HALF:, :] if k >= 2 else z3_d[:, :]
                b_tab = z3_d[HALF:, :] if k % 2 else z3_d[:, :]
                for s0 in range(0, tk, DEC_SC):
                    scn = min(DEC_SC, tk - s0)
                    toff = tbase + s0
                    ga = dp.tile([P, DEC_SC * out_c], f32, tag="ga")
                    gb = dp.tile([P, DEC_SC * out_c], f32, tag="gb")
                    emit_gather(ga[:, :scn * out_c].rearrange(
                        "p (c f) -> p c f", c=scn), a_tab, la_sb,
                        toff, 0, scn, out_c)
                    emit_gather(gb[:, :scn * out_c].rearrange(
                        "p (c f) -> p c f", c=scn), b_tab, lb_sb,
                        toff, 0, scn, out_c)
                    nc.vector.tensor_tensor(
                        out=ga[:, :scn * out_c], in0=ga[:, :scn * out_c],
                        in1=gb[:, :scn * out_c], op=mybir.AluOpType.mult)
                    nc.vector.tensor_reduce(
                        out=res[:, toff:toff + scn],
                        in_=ga[:, :scn * out_c].rearrange(
                            "p (c f) -> p c f", c=scn),
                        axis=mybir.AxisListType.X, op=mybir.AluOpType.add)
                tbase += tk
            nc.sync.dma_start(out_d[:, :], res[:])

    nc.finalize()
    return nc


# ---------------------------------------------------------------- entry point

def kernel(x, W1, b1, W2, b2, W3, b3, edge_index, edge_label_index):
    from concourse.bass_utils import run_bass_kernel_spmd

    x = np.ascontiguousarray(np.asarray(x, dtype=np.float32))
    n_nodes, in_c = x.shape
    hid_c = np.asarray(W2).shape[0]
    out_c = np.asarray(W3).shape[1]
    n_label = np.asarray(edge_label_index).shape[1]
    bpc = int(np.ceil(n_nodes / (N_CORES * P)))
    npad = N_CORES * bpc * P
    nrows = HEAD + npad + P

    ed = prepare_edges(edge_index, n_nodes, bpc)
    lb = prepare_labels(edge_label_index, n_label, ed["rowmap"])

    bias_zero = all(np.all(np.asarray(b) == 0) for b in (b1, b2, b3))
    nc = build_bass(n_nodes, bpc, ed["groups"], ed["sli"], ed["shi"],
                    ed["nsl"], ed["nsh"], lb["tcnt"], in_c, hid_c, out_c,
                    bias_zero=bias_zero)

    # host-side layer-0 table: x * dinv, bf16, group-major rows, zero pads
    xp = np.zeros((npad, in_c), np.float32)
    xp[:n_nodes] = x
    xt = np.zeros((nrows, in_c), ml_dtypes.bfloat16)
    xt[ed["rowmap"]] = (xp * ed["dinv"][:, None]).astype(ml_dtypes.bfloat16)

    common = {
        "xt": xt,
        "W1": np.asarray(W1, np.float32).astype(ml_dtypes.bfloat16),
        "W2": np.asarray(W2, np.float32).astype(ml_dtypes.bfloat16),
        "W3": np.asarray(W3, np.float32).astype(ml_dtypes.bfloat16),
        "b1": np.ascontiguousarray(np.asarray(b1, np.float32)),
        "b2": np.ascontiguousarray(np.asarray(b2, np.float32)),
        "b3": np.ascontiguousarray(np.asarray(b3, np.float32)),
    }
    in_maps = []
    for c in range(N_CORES):
        m = dict(common)
        m["eidx"] = np.ascontiguousarray(ed["eidx"][c])
        m["spidx"] = np.ascontiguousarray(
            ed["spidx"][c] if ed["nsl"] + ed["nsh"] else
            np.zeros((P, 16), np.int16))
        m["sind"] = np.ascontiguousarray(ed["sind"][c])
        m["dinv1"] = np.ascontiguousarray(ed["dinv1"][c])
        m["dinv2"] = np.ascontiguousarray(ed["dinv2"][c])
        m["la"] = np.ascontiguousarray(lb["la"][c])
        m["lb"] = np.ascontiguousarray(lb["lb"][c])
        in_maps.append(m)

    res = run_bass_kernel_spmd(nc, in_maps, core_ids=list(range(N_CORES)))

    out = np.zeros((n_label,), np.float32)
    for c in range(N_CORES):
        o = res.results[c]["out"]  # [P, T]
        flat = o.T.reshape(-1)
        lm = lb["labmap"][c]
        valid = lm >= 0
        out[lm[valid]] = flat[valid]
    return out
